# revision 7
# baseline (speedup 1.0000x reference)
# GQA causal attention with RoPE on 8 TRN2 NeuronCores (tensor-parallel over
# heads) -- fused-pipeline version.
#
# Reference computation (B=2, S=4096, D=2048, H=16 heads, KVH=4 kv heads,
# HD=128): q/k/v projections -> RoPE on q,k -> causal GQA attention -> o_proj.
#
# Sharding (per hint): core c owns Q heads {2c, 2c+1}; kv head c//2 is split
# across the core pair -- the even core projects K (with RoPE), the odd core
# projects V (same SPMD program: its RoPE tables are cos=1/sin=0), and a
# pairwise AllGather swaps the halves per sequence-half.  Attention context
# is produced transposed [HD, B*S] per head; a per-head AllToAll
# redistributes rows for the o_proj row shard.
#
# This version software-pipelines the WHOLE kernel as one flat emission
# stream so the PE (the global bottleneck at ~560us of matmul work) never
# idles:
#   - batch-0 projections run first (PE-solid), shipping each K/V
#     sequence-half to the pair exchange as soon as it completes;
#   - batch-1 projections are chopped into 8-matmul chunks and interleaved
#     one-per-task into attention section (h0,b0) -- the exp latency of task
#     t is hidden behind the chunk emitted before scores(t+1);
#   - the o_proj even-head pass (which only needs the h0 AllToAll, complete
#     at mid-attention) is interleaved into the ACT-bound sections (h1,b0/b1)
#     and staged to SBUF, so only the odd-head pass remains after the last
#     attention task;
#   - PSUM is juggled via dual-side pool stacks: the projection pools live on
#     the right stack and release mid-stream, after which the score pipeline
#     gets its second buffer.
#
# Causal masking costs no vector work: a -1e9 strict-upper-triangle is added
# to diagonal 128-blocks inside the score-PSUM accumulation group by a
# 128-col matmul, so exp underflows to exact 0 there; fully-masked ranges of
# the dedicated diagonal et tiles are memzero'd once.  The softmax
# denominator accumulates per kv-block on the DVE in bf16 and is collapsed
# AND broadcast in one step by a gpsimd partition_all_reduce (the gpsimd
# engine is otherwise idle), so the normalize path has no PE matmuls at all.
# RoPE stages the projection PSUM through bf16 SBUF (scalar-engine copy plus
# two gpsimd partition-swapped copies for rotate_half), which frees the
# projection PSUM ring quickly and lets every RoPE DVE op run same-base at
# the 2x 16-bit rate.  V is transposed to natural layout on the DMA XBAR.
# h1's context AllToAll is split into even/odd sequence-parity halves so the
# even half is exchanged one q-block before attention ends and the o_proj
# tail starts immediately.  Matmul operands are bf16; PSUM f32; the output
# is staged bf16 (host converts to f32).

import math
import sys

for _p in ("/opt/trn_rl_repo",):
    if _p not in sys.path:
        sys.path.insert(0, _p)

import numpy as np
import ml_dtypes

B = 2
S = 4096
D = 2048
H = 16
KVH = 4
HD = 128
N_CORES = 8
BS = B * S                  # 8192 flattened rows
SHARD = BS // N_CORES       # 1024 output rows per core
HPC = H // N_CORES          # 2 q heads per core
SCALE = 1.0 / math.sqrt(HD)

SQ = 512                    # q-block (matmul free dim)
KV = 128                    # kv-block (psum partition dim)
DCH = D // 128              # 16 contraction chunks for the projections
NB = S // SQ                # 8 q-blocks per batch
NKV_B = S // KV             # 32 kv-blocks per batch
DIAG = SQ // KV             # 4 kv-blocks per q-block on the causal diagonal
S2 = S // 2

EV_N = 16                   # o_proj even-pass tiles prestaged during attention

BF16 = ml_dtypes.bfloat16

_CACHE = {}
PHASE_MARKS = []
EMIT_LOG = []


def _mark(nc, phase):
    try:
        PHASE_MARKS.append((phase, int(nc._state.next_id())))
    except Exception:
        pass


def _build(sim_mode=False):
    import concourse.mybir as mybir
    import concourse.tile as tile
    from concourse import bacc

    dt = mybir.dt
    nc = bacc.Bacc("TRN2", target_bir_lowering=False, debug=False,
                   enable_asserts=True, num_devices=N_CORES)

    # ---- external inputs (per-core shards supplied via in_maps) ----
    xT = nc.dram_tensor("xT", [D, BS], dt.bfloat16, kind="ExternalInput")
    cosT = nc.dram_tensor("cosT", [HD, S], dt.bfloat16, kind="ExternalInput")
    sinTs = nc.dram_tensor("sinTs", [HD, S], dt.bfloat16, kind="ExternalInput")
    wq = nc.dram_tensor("wq", [D, HPC * HD], dt.bfloat16, kind="ExternalInput")
    wkv = nc.dram_tensor("wkv", [D, HD], dt.bfloat16, kind="ExternalInput")
    coskv = nc.dram_tensor("coskv", [HD, S], dt.bfloat16, kind="ExternalInput")
    sinkv = nc.dram_tensor("sinkv", [HD, S], dt.bfloat16, kind="ExternalInput")
    wo = nc.dram_tensor("wo", [D, D], dt.bfloat16, kind="ExternalInput")
    tri = nc.dram_tensor("tri", [128, 128], dt.bfloat16, kind="ExternalInput")
    ident = nc.dram_tensor("ident", [128, 128], dt.bfloat16, kind="ExternalInput")
    onesb = nc.dram_tensor("onesb", [128, 1], dt.bfloat16, kind="ExternalInput")
    onesf = nc.dram_tensor("onesf", [1, 128], dt.bfloat16, kind="ExternalInput")

    out = nc.dram_tensor("out", [SHARD, D], dt.bfloat16, kind="ExternalOutput")

    # ---- internal DRAM for the pairwise k/v exchange (per sequence-half) ----
    ktv_dram = [nc.dram_tensor(f"ktv{b}", [2, HD, S2], dt.bfloat16)
                for b in range(B)]
    kv_pair = [nc.dram_tensor(f"kvp{b}", [2, 2, HD, S2], dt.bfloat16)
               for b in range(B)]

    # ---- internal DRAM for the AllToAll: one buffer for h0, and h1 split
    # into even/odd sequence-parity halves so the even collective fires one
    # q-block before the end of attention ----
    ao_in = [nc.dram_tensor(f"ao_in{h}", [N_CORES, HD, SHARD], dt.bfloat16)
             for h in range(HPC)]
    ao_ex = [nc.dram_tensor(f"ao_ex{h}", [N_CORES, HD, SHARD], dt.bfloat16)
             for h in range(HPC)]
    ao1_in = [nc.dram_tensor(f"ao1_in{p}", [N_CORES, HD, SQ], dt.bfloat16)
              for p in range(2)]
    ao1_ex = [nc.dram_tensor(f"ao1_ex{p}", [N_CORES, HD, SQ], dt.bfloat16)
              for p in range(2)]
    if sim_mode:
        ao_ex = ao_in   # single-core TimelineSim: same DMA pattern
        ao1_ex = ao1_in

    with tile.TileContext(nc) as tc:
        # -------- long-lived pools (left stacks) --------
        pp = tc.alloc_tile_pool(name="persist", bufs=1)
        qkvp = tc.alloc_tile_pool(name="qkv", bufs=2)
        pbe = tc.alloc_tile_pool(name="pbe", bufs=5)
        pbd = tc.alloc_tile_pool(name="pbd", bufs=1)
        pbn = tc.alloc_tile_pool(name="pbn", bufs=4)
        pbsc_a = tc.alloc_tile_pool(name="pbsca", bufs=1, space="PSUM")
        pbsc_b = tc.alloc_tile_pool(name="pbscb", bufs=1, space="PSUM")
        pbo = tc.alloc_tile_pool(name="pbo", bufs=2, space="PSUM")
        # -------- projection-phase pools (right stacks; close mid-stream) ----
        # vtb sits below pA on the right stack: pA releases first (after the
        # last RoPE) so the score pipeline gets its second PSUM buffer while
        # the V transposes are still reading vtb.
        pvtb = tc.alloc_tile_pool(name="pvtb", bufs=1, side="right")
        pA = tc.alloc_tile_pool(name="pA", bufs=1, side="right")
        pAps = tc.alloc_tile_pool(name="pAps", bufs=2, side="right",
                                  space="PSUM")

        tri_sb = pp.tile([128, 128], dt.bfloat16, name="tri_sb")
        id_sb = pp.tile([128, 128], dt.bfloat16, name="id_sb")
        ob_sb = pp.tile([128, 1], dt.bfloat16, name="ob_sb")
        of_sb = pp.tile([1, 128], dt.bfloat16, name="of_sb")

        cos_sb = pA.tile([HD, S], dt.bfloat16, name="cos_sb")
        sin_sb = pA.tile([HD, S], dt.bfloat16, name="sin_sb")
        ckv_sb = pA.tile([HD, S], dt.bfloat16, name="ckv_sb")
        skv_sb = pA.tile([HD, S], dt.bfloat16, name="skv_sb")
        wq_sb = pA.tile([128, DCH, HPC * HD], dt.bfloat16, name="wq_sb")
        wkv_sb = pA.tile([128, DCH, HD], dt.bfloat16, name="wkv_sb")
        vtb = pvtb.tile([HD, S], dt.bfloat16, name="vtb")

        # first contraction chunks ahead of the rest so the opening matmuls
        # aren't stuck behind the full weight DMA
        wqr = wq[:].rearrange("(k p) m -> p k m", p=128)
        wkvr = wkv[:].rearrange("(k p) m -> p k m", p=128)
        nc.sync.dma_start(out=wq_sb[:, 0:2, :], in_=wqr[:, 0:2, :])
        nc.sync.dma_start(out=wkv_sb[:, 0:2, :], in_=wkvr[:, 0:2, :])

        qts, kts, vns = {}, {}, {}
        xs_state = {}
        ktv_tiles = {}
        tpose_state = {}
        stg = {}
        woq_t = {}
        lt_all = {}
        mid = {}          # pools opened mid-stream

        mybir_exp = mybir.ActivationFunctionType.Exp

        def get_qt(b):
            if b not in qts:
                qts[b] = qkvp.tile([HD, HPC, S], dt.bfloat16, name=f"qt{b}",
                                   tag="qt")
            return qts[b]

        def get_kt(b):
            if b not in kts:
                kts[b] = qkvp.tile([HD, S], dt.bfloat16, name=f"kt{b}",
                                   tag="kt")
            return kts[b]

        def get_vn(b):
            if b not in vns:
                vns[b] = qkvp.tile([128, NKV_B, HD], dt.bfloat16,
                                   name=f"vn{b}", tag="vn")
            return vns[b]

        def load_xs(b, sj, split=2):
            t = pA.tile([128, DCH, SQ], dt.bfloat16, name="xs", tag="xs",
                        bufs=2)
            xr = xT[:, b * S + sj * SQ:b * S + sj * SQ + SQ].rearrange(
                "(k p) n -> p k n", p=128)
            step = DCH // split
            for h0 in range(0, DCH, step):
                nc.sync.dma_start(out=t[:, h0:h0 + step, :],
                                  in_=xr[:, h0:h0 + step, :])
            xs_state[(b, sj)] = t
            return t

        # ---------------- A: projection chunks ----------------
        # One pass per projection (q-head0 / q-head1 / k-or-v), 16 matmuls
        # into a single [128,SQ] PSUM tile (ring 2), emitted as two 8-matmul
        # chunks so interleaved B tasks see fine-grained PE filler.
        a_psum = {}

        def emit_A(b, si, kind, half):
            if kind == 0 and half == 0:
                if b == 0 and si == 0:
                    # feed the serial DMA queue in the exact order the first
                    # projection passes consume: (weight chunk pair, xs
                    # eighth) pairs in k order, with the rope-table halves
                    # slotted in where the first RoPE needs them
                    t = pA.tile([128, DCH, SQ], dt.bfloat16, name="xs",
                                tag="xs", bufs=2)
                    xr = xT[:, 0:SQ].rearrange("(k p) n -> p k n", p=128)
                    for k8 in range(8):
                        nc.sync.dma_start(out=t[:, 2 * k8:2 * k8 + 2, :],
                                          in_=xr[:, 2 * k8:2 * k8 + 2, :])
                        if k8 < 7:
                            nc.sync.dma_start(
                                out=wq_sb[:, 2 * k8 + 2:2 * k8 + 4, :],
                                in_=wqr[:, 2 * k8 + 2:2 * k8 + 4, :])
                        if k8 == 6:
                            nc.sync.dma_start(out=cos_sb[:, 0:S // 4],
                                              in_=cosT[:, 0:S // 4])
                            nc.sync.dma_start(out=sin_sb[:, 0:S // 4],
                                              in_=sinTs[:, 0:S // 4])
                    xs_state[(0, 0)] = t
                    nc.sync.dma_start(out=wkv_sb[:, 2:DCH, :],
                                      in_=wkvr[:, 2:DCH, :])
                    nc.sync.dma_start(out=ckv_sb[:, 0:S // 4],
                                      in_=coskv[:, 0:S // 4])
                    nc.sync.dma_start(out=skv_sb[:, 0:S // 4],
                                      in_=sinkv[:, 0:S // 4])
                    nc.sync.dma_start(out=tri_sb[:], in_=tri[:])
                    nc.sync.dma_start(out=id_sb[:], in_=ident[:])
                    nc.sync.dma_start(out=ob_sb[:], in_=onesb[:])
                    nc.sync.dma_start(out=of_sb[:], in_=onesf[:])
                    # warm the exp table set while the scalar engine is idle
                    warm = pA.tile([1, 8], dt.bfloat16, name="warm")
                    nc.scalar.activation(warm[:], tri_sb[0:1, 0:8],
                                         mybir_exp, scale=1.0)
                if (b, si) not in xs_state:
                    load_xs(b, si, split=2)
                if b == 0 and si == 1:
                    nc.sync.dma_start(out=cos_sb[:, S // 4:S2],
                                      in_=cosT[:, S // 4:S2])
                    nc.sync.dma_start(out=sin_sb[:, S // 4:S2],
                                      in_=sinTs[:, S // 4:S2])
                    nc.sync.dma_start(out=ckv_sb[:, S // 4:S2],
                                      in_=coskv[:, S // 4:S2])
                    nc.sync.dma_start(out=skv_sb[:, S // 4:S2],
                                      in_=sinkv[:, S // 4:S2])
                if b == 0 and si == 2:
                    nc.sync.dma_start(out=cos_sb[:, S2:S], in_=cosT[:, S2:S])
                    nc.sync.dma_start(out=sin_sb[:, S2:S], in_=sinTs[:, S2:S])
                    nc.sync.dma_start(out=ckv_sb[:, S2:S],
                                      in_=coskv[:, S2:S])
                    nc.sync.dma_start(out=skv_sb[:, S2:S],
                                      in_=sinkv[:, S2:S])
                # prefetch the next activation block
                nxt = (b, si + 1) if si + 1 < NB else (b + 1, 0)
                if nxt[0] < B and nxt not in xs_state:
                    load_xs(*nxt)
            if kind == 2 and half == 0 and si % (NB // 2) == 0:
                ktv_tiles[(b, si // (NB // 2))] = pA.tile(
                    [HD, S2], dt.bfloat16, name=f"ktv{b}", tag="ktv", bufs=2)
            xs = xs_state[(b, si)]
            if half == 0:
                a_psum[(b, si, kind)] = pAps.tile([128, SQ], dt.float32,
                                                  name="pp", tag="pp")
            pt = a_psum[(b, si, kind)]
            for k in range(half * 8, half * 8 + 8):
                if kind < 2:
                    lhsT = wq_sb[:, k, kind * HD:(kind + 1) * HD]
                else:
                    lhsT = wkv_sb[:, k, :]
                nc.tensor.matmul(pt[:], lhsT=lhsT, rhs=xs[:, k, :],
                                 start=(k == 0), stop=(k == DCH - 1))
            if half == 1:
                ph = a_psum.pop((b, si, kind))
                l0 = si * SQ
                if kind < 2:
                    cs, sn = cos_sb, sin_sb
                    dest = get_qt(b)[:, kind, l0:l0 + SQ]
                else:
                    cs, sn = ckv_sb, skv_sb
                    lh = l0 % S2
                    dest = ktv_tiles[(b, si // (NB // 2))][:, lh:lh + SQ]
                # stage the projection through bf16 SBUF: a scalar-engine
                # copy (straight) plus two gpsimd copies (partition-swapped
                # halves for rotate_half -- the DVE cannot read two SBUF
                # operands at different base partitions).  All RoPE DVE ops
                # then run same-base at the 2x 16-bit rate, and the PSUM slot
                # frees without any DVE work.
                phb = pA.tile([128, SQ], dt.bfloat16, name="phb", tag="phb",
                              bufs=3)
                phs = pA.tile([128, SQ], dt.bfloat16, name="phs", tag="phs",
                              bufs=3)
                nc.scalar.copy(out=phb[:], in_=ph[:])
                nc.gpsimd.tensor_copy(out=phs[0:64, :], in_=phb[64:128, :])
                nc.gpsimd.tensor_copy(out=phs[64:128, :], in_=phb[0:64, :])
                t1 = pA.tile([128, SQ], dt.bfloat16, name="t1", tag="t1",
                             bufs=2)
                t2 = pA.tile([128, SQ], dt.bfloat16, name="t2", tag="t2",
                             bufs=2)
                nc.vector.tensor_mul(out=t1[:], in0=phb[:],
                                     in1=cs[:, l0:l0 + SQ])
                nc.vector.tensor_mul(out=t2[:], in0=phs[:],
                                     in1=sn[:, l0:l0 + SQ])
                nc.vector.tensor_add(out=dest, in0=t1[:], in1=t2[:])

        def emit_ship(b, hx):
            # ship a completed sequence-half of this core's k-or-v and start
            # the pair exchange
            nc.sync.dma_start(out=ktv_dram[b][hx], in_=ktv_tiles[(b, hx)])
            if not sim_mode:
                nc.gpsimd.collective_compute(
                    "AllGather", mybir.AluOpType.bypass,
                    replica_groups=[[2 * g, 2 * g + 1]
                                    for g in range(N_CORES // 2)],
                    ins=[ktv_dram[b][hx]],
                    outs=[kv_pair[b][hx]])
            else:
                nc.sync.dma_start(out=kv_pair[b][hx, 0], in_=ktv_dram[b][hx])
                nc.sync.dma_start(out=kv_pair[b][hx, 1], in_=ktv_dram[b][hx])

        def emit_xrb(b, hx, part=2):
            # pull back this core's K half and/or V half from the exchange
            if part in (0, 2):
                nc.sync.dma_start(out=get_kt(b)[:, hx * S2:(hx + 1) * S2],
                                  in_=kv_pair[b][hx, 0])
            if part in (1, 2):
                nc.sync.dma_start(out=vtb[:, hx * S2:(hx + 1) * S2],
                                  in_=kv_pair[b][hx, 1])

        def emit_T(b, c):
            # transpose 8 kv-blocks of V from [HD, kv] to natural [kv, HD]
            # on the DMA XBAR (no PE/ACT/PSUM involvement)
            vnb = get_vn(b)
            nc.sync.dma_start_transpose(
                vnb[:, c * 8:c * 8 + 8, :],
                vtb[:, c * 8 * 128:(c * 8 + 8) * 128])

        # ---------------- B: attention machinery ----------------
        sections = [(h, b) for h in range(HPC) for b in range(B)]
        tasks = []
        for sidx, (h, b) in enumerate(sections):
            for si in range(NB):
                for j2 in range((si + 1) * DIAG // 2):
                    tasks.append((sidx, si, j2))
        # guest tasks: the first four q-block rows of (h1,b0) only need the
        # half-0 K/V exchange, so they run inside A0's window where the
        # scalar engine is otherwise idle -- their exp leaves the ACT-paced
        # sections entirely
        guest = [t for t in tasks if t[0] == 2 and t[1] < 4]
        tasks = guest + [t for t in tasks if t not in guest]
        bidx = {t: i for i, t in enumerate(tasks)}

        # dedicated diagonal-pair et tiles: fully-masked column ranges zeroed
        # ONCE (exp only writes the live ranges)
        etdAs, etdBs = [], []
        for r in range(2):
            etdA = pbd.tile([128, 2 * SQ], dt.bfloat16, name=f"etdA{r}")
            etdB = pbd.tile([128, 2 * SQ], dt.bfloat16, name=f"etdB{r}")
            nc.vector.memzero(etdA[:, SQ:SQ + KV])
            nc.vector.memzero(etdB[:, 0:2 * KV])
            nc.vector.memzero(etdB[:, SQ:SQ + 3 * KV])
            etdAs.append(etdA)
            etdBs.append(etdB)
        POOLS = [pbsc_a, pbsc_b]
        psc_of = {}
        psc_n = [0]

        def emit_scores(t):
            sidx, si, j2 = t
            h, b = sections[sidx]
            pool = POOLS[psc_n[0] % len(POOLS)]
            psc_n[0] += 1
            psc = pool.tile([128, 2 * SQ], dt.float32, name="psc", tag="psc")
            qt, kt = qts[b], kts[b]
            ndiag = si * DIAG
            for jj in range(2):
                j = j2 * 2 + jj
                dd = j - ndiag
                half = jj * SQ
                nc.tensor.matmul(
                    psc[:, half:half + SQ],
                    lhsT=kt[:, j * KV:(j + 1) * KV],
                    rhs=qt[:, h, si * SQ:(si + 1) * SQ],
                    start=True, stop=(dd < 0))
                if dd >= 0:
                    # strict-upper -1e9 on the diagonal block: exp -> exact 0
                    nc.tensor.matmul(
                        psc[:, half + dd * KV:half + (dd + 1) * KV],
                        lhsT=tri_sb[:], rhs=id_sb[:],
                        start=False, stop=True, skip_group_check=True)
            psc_of[t] = psc

        def emit_exp(t, psc):
            sidx, si, j2 = t
            ndiag2 = si * DIAG // 2
            if j2 < ndiag2:
                et = pbe.tile([128, 2 * SQ], dt.bfloat16, name="et", tag="et")
                nc.scalar.activation(et[:], psc[:], mybir_exp, scale=SCALE)
            elif j2 == ndiag2:          # diagonal pair A (dd=0,1)
                et = etdAs[si % 2]
                nc.scalar.activation(et[:, 0:SQ], psc[:, 0:SQ],
                                     mybir_exp, scale=SCALE)
                nc.scalar.activation(et[:, SQ + KV:2 * SQ],
                                     psc[:, SQ + KV:2 * SQ],
                                     mybir_exp, scale=SCALE)
            else:                       # diagonal pair B (dd=2,3)
                et = etdBs[si % 2]
                nc.scalar.activation(et[:, 2 * KV:SQ], psc[:, 2 * KV:SQ],
                                     mybir_exp, scale=SCALE)
                nc.scalar.activation(et[:, SQ + 3 * KV:2 * SQ],
                                     psc[:, SQ + 3 * KV:2 * SQ],
                                     mybir_exp, scale=SCALE)
            return et

        def emit_acc(t, et, acc):
            _, si, j2 = t
            if j2 == 0:
                nc.vector.tensor_add(out=acc[:], in0=et[:, 0:SQ],
                                     in1=et[:, SQ:2 * SQ])
            else:
                nc.vector.tensor_add(out=acc[:], in0=acc[:], in1=et[:, 0:SQ])
                nc.vector.tensor_add(out=acc[:], in0=acc[:],
                                     in1=et[:, SQ:2 * SQ])

        def emit_av(t, et, po):
            sidx, si, j2 = t
            h, b = sections[sidx]
            vn = vns[b]
            nkv = (si + 1) * DIAG
            for jj in range(2):
                j = j2 * 2 + jj
                nc.tensor.matmul(po[:], lhsT=vn[:, j, :],
                                 rhs=et[:, jj * SQ:(jj + 1) * SQ],
                                 start=(j == 0), stop=(j == nkv - 1))

        # deferred normalize: stage1 (ones-matmul + reciprocal) in the next
        # q-block's first pair window, stage2 (broadcast matmul + PSUM-direct
        # multiply + per-q-block aob ship) in the second
        from concourse import bass_isa

        def norm1(po_, acc_, hh, bb, si):
            # partition-ALL-reduce of the bf16 denominator accumulator on the
            # otherwise-idle gpsimd engine: every partition gets the sum, so
            # no ones-matmul and no broadcast matmul are needed
            dall = pbn.tile([128, SQ], dt.float32, name="dall", tag="dall",
                            bufs=2)
            nc.gpsimd.partition_all_reduce(dall[:], acc_[:], channels=128,
                                           reduce_op=bass_isa.ReduceOp.add)
            rec = pbn.tile([128, SQ], dt.float32, name="rec", tag="rec",
                           bufs=2)
            nc.vector.reciprocal(out=rec[:], in_=dall[:])
            return rec

        def norm2(po_, acc_, hh, bb, si, rec):
            aob = pbn.tile([HD, SQ], dt.bfloat16, name="aob", tag="aob",
                           bufs=3)
            nc.vector.tensor_mul(out=aob[:], in0=po_[:], in1=rec[:])
            g0 = bb * S + si * SQ
            if hh == 0:
                nc.sync.dma_start(
                    out=ao_in[0][g0 // SHARD, :, g0 % SHARD:g0 % SHARD + SQ],
                    in_=aob[:])
                if bb == B - 1 and si == NB - 1 and not sim_mode:
                    nc.gpsimd.collective_compute(
                        "AllToAll", mybir.AluOpType.bypass,
                        replica_groups=[list(range(N_CORES))],
                        ins=[ao_in[0][:]],
                        outs=[ao_ex[0][:]])
            else:
                par = si % 2
                nc.sync.dma_start(
                    out=ao1_in[par][bb * 4 + si // 2, :, :], in_=aob[:])
                if bb == B - 1 and si >= NB - 2 and not sim_mode:
                    nc.gpsimd.collective_compute(
                        "AllToAll", mybir.AluOpType.bypass,
                        replica_groups=[list(range(N_CORES))],
                        ins=[ao1_in[par][:]],
                        outs=[ao1_ex[par][:]])

        # ---------------- D: o_proj machinery ----------------
        def emit_opend():
            mid["plt"] = tc.alloc_tile_pool(name="plt", bufs=1)
            mid["pdw"] = tc.alloc_tile_pool(name="pdw", bufs=2)
            mid["pstg"] = tc.alloc_tile_pool(name="pstg", bufs=EV_N)
            mid["pdot"] = tc.alloc_tile_pool(name="pdot", bufs=4)

        def emit_lt(h, part=2):
            if h == 0:
                lt = mid["plt"].tile([128, SHARD // 128, N_CORES, 128],
                                     dt.bfloat16, name="lt0")
                lt_all[h] = lt
                nc.sync.dma_start(
                    out=lt[:],
                    in_=ao_ex[0][:].rearrange("a p (s n) -> p s a n", n=128))
                return
            # per-si chunks from the parity halves: chunks 0-3 (even parity)
            # are exchanged one q-block before the last ship, so their loads
            # are emitted during the last attention q-block
            if 1 not in lt_all:
                lt_all[1] = mid["plt"].tile(
                    [128, SHARD // 128, N_CORES, 128], dt.bfloat16,
                    name="lt1")
            lt = lt_all[1]
            sls = range(0, 4) if part == 0 else range(4, 8)
            for sl in sls:
                par, c0 = sl // 4, (sl % 4) * 128
                nc.sync.dma_start(
                    out=lt[:, sl],
                    in_=ao1_ex[par][:, :, c0:c0 + 128]
                    .rearrange("a p n -> p a n"))

        def emit_woq(dj, split):
            t = mid["pdw"].tile([128, DCH, SQ], dt.bfloat16, name="woq",
                                tag="woq")
            r = wo[:, dj * SQ:(dj + 1) * SQ].rearrange("(k p) m -> p k m",
                                                       p=128)
            if split:
                # even k chunks first (the even-head pass uses k=2j)
                nc.sync.dma_start(out=t[:, 0:DCH:2, :], in_=r[:, 0:DCH:2, :])
                nc.sync.dma_start(out=t[:, 1:DCH:2, :], in_=r[:, 1:DCH:2, :])
            else:
                nc.sync.dma_start(out=t[:], in_=r)
            woq_t[dj] = t

        def emit_even(dj, sl):
            # even-head half of o_proj tile (dj, sl), staged to SBUF f32 so
            # the PSUM bank recycles; the odd pass adds it back in the tail
            pev = mid["paux"].tile([128, SQ], dt.float32, name="pev",
                                   tag="evn")
            for j in range(N_CORES):
                nc.tensor.matmul(pev[:], lhsT=lt_all[0][:, sl, j, :],
                                 rhs=woq_t[dj][:, 2 * j, :],
                                 start=(j == 0), stop=(j == N_CORES - 1))
            st = mid["pstg"].tile([128, SQ], dt.float32, name="stg",
                                  tag="stg")
            nc.vector.tensor_copy(out=st[:], in_=pev[:])
            stg[(dj, sl)] = st

        tail_n = [0]

        def emit_tail(dj, sl):
            # odd-head pass (+ even remainder) for o_proj tile (dj, sl);
            # po slots alternate between the two PSUM pools (ring 4) so the
            # tail is never blocked behind the final normalize chain
            tail_n[0] += 1
            if tail_n[0] % 2 == 0:
                pod = mid["paux"].tile([HD, SQ], dt.float32, name="po",
                                       tag="evn")
            else:
                pod = pbo.tile([HD, SQ], dt.float32, name="po", tag="po")
            pre = stg.get((dj, sl))
            hps = (1,) if pre is not None else (0, 1)
            for hp in hps:
                for j in range(N_CORES):
                    nc.tensor.matmul(
                        pod[:], lhsT=lt_all[hp][:, sl, j, :],
                        rhs=woq_t[dj][:, 2 * j + hp, :],
                        start=(hp == hps[0] and j == 0),
                        stop=(hp == hps[-1] and j == N_CORES - 1))
            ot = mid["pdot"].tile([128, SQ], dt.bfloat16, name="ot", tag="ot")
            if pre is not None:
                nc.vector.tensor_add(out=ot[:], in0=pod[:], in1=pre[:])
            else:
                nc.scalar.copy(out=ot[:], in_=pod[:])
            nc.sync.dma_start(
                out=out[sl * 128:(sl + 1) * 128, dj * SQ:(dj + 1) * SQ],
                in_=ot[:])

        # ---------------- schedule assembly ----------------
        a_chunks = {b: [(b, si, kind, half)
                        for si in range(NB)
                        for kind in range(3)
                        for half in range(2)]
                    for b in range(B)}

        items = []
        # batch-0 projections, PE-solid, shipping halves as they complete;
        # once the half-0 exchange is back (si4) the V half transposes land
        # and the guest (h1,b0) tasks interleave with the si5-7 chunks
        gi = 0
        for ch in a_chunks[0]:
            b, si, kind, half = ch
            if si == 5 and kind == 0 and half == 0:
                items.append(("T", 0, 0))
                items.append(("T", 0, 1))
            items.append(("A",) + ch)
            if si >= 5 and gi < len(guest):
                items.append(("B", guest[gi]))
                gi += 1
            if kind == 2 and half == 1 and si in (NB // 2 - 1, NB - 1):
                items.append(("SHIP", 0, si // (NB // 2)))
                items.append(("XRB", 0, si // (NB // 2)))
        while gi < len(guest):
            items.append(("B", guest[gi]))
            gi += 1

        # section (0,0) with batch-1 projection chunks interleaved one per
        # task (done by task 48), then T(b1) and the psc second buffer
        sec_tasks = [[t for t in tasks if t[0] == s and (s != 2 or t[1] >= 4)]
                     for s in range(4)]
        a1 = list(a_chunks[1])
        merged = []
        ai = 0
        for ti, t in enumerate(sec_tasks[0]):
            if ai < len(a1):
                merged.append(("A",) + a1[ai])
                b, si, kind, half = a1[ai]
                if kind == 2 and half == 1 and si in (NB // 2 - 1, NB - 1):
                    merged.append(("SHIP", 1, si // (NB // 2)))
                ai += 1
            if ti in (2, 4):
                merged.append(("T", 0, ti // 2 + 1))
            if ti == 27:
                merged.append(("XRB", 1, 0, 0))
            if ti == 29:
                merged.append(("XRB", 1, 0, 1))
            if ti in (32, 34):
                merged.append(("T", 1, (ti - 32) // 2))
            if ti == 48:
                merged.append(("CLOSEA",))
                merged.append(("XRB", 1, 1))
            if ti in (50, 52):
                merged.append(("T", 1, (ti - 50) // 2 + 2))
            if ti == 58:
                merged.append(("CLOSEVTB",))
            merged.append(("B", t))
        items += merged
        # section (0,1): mostly pure attention; prefetch the o_proj weights
        # here where the serial DMA queue is quiet
        for ti, t in enumerate(sec_tasks[1]):
            if ti == 8:
                items.append(("OPEND",))
                items.append(("WOQ", 0, True))
            if ti == 40:
                items.append(("WOQ", 1, False))
            items.append(("B", t))
        # section (1,0): open o_proj pools once h0's AllToAll has fired
        # (inside the norm2 of (0,1)'s last q-block, processed at task 1)
        ev_slots0 = (8, 17, 26, 35, 44)
        done_ev = 0
        for ti, t in enumerate(sec_tasks[2]):
            if ti == 4:
                items.append(("LT0",))
            if ti in ev_slots0 and done_ev < EV_N:
                items.append(("EV", done_ev // 8, done_ev % 8))
                done_ev += 1
            items.append(("B", t))
        # section (1,1): more even-pass tiles in the ACT-bound slack
        ev_slots = (10, 22, 34, 46, 58)
        for ti, t in enumerate(sec_tasks[3]):
            if ti in ev_slots and done_ev < EV_N:
                items.append(("EV", done_ev // 8, done_ev % 8))
                done_ev += 1
            if ti == 66:
                items.append(("LT1A",))
            items.append(("B", t))
        # tail: flush the last norm, land h1 context; the last even-pass
        # tiles (lt0-only) fill the flush->lt1 latency
        items.append(("FLUSH",))
        items.append(("LT1",))
        while done_ev < EV_N:
            items.append(("EV", done_ev // 8, done_ev % 8))
            done_ev += 1
        # even-parity rows first: their h1 exchange fired one q-block early
        for sl in range(4):
            items.append(("TAIL", 0, sl))
        for sl in range(4):
            items.append(("TAIL", 1, sl))
        for sl in range(4, 8):
            items.append(("TAIL", 0, sl))
        items.append(("WOQ", 2, False))
        for sl in range(4, 8):
            items.append(("TAIL", 1, sl))
        items.append(("WOQ", 3, False))
        for sl in range(8):
            items.append(("TAIL", 2, sl))
        for sl in range(8):
            items.append(("TAIL", 3, sl))

        # ---------------- executor ----------------
        state = {"pending": None, "po": None, "acc": None}

        def run_filler(it):
            kind = it[0]
            _log(str(it))
            if kind == "A":
                emit_A(*it[1:])
            elif kind == "SHIP":
                emit_ship(*it[1:])
            elif kind == "XRB":
                emit_xrb(*it[1:])
            elif kind == "T":
                emit_T(*it[1:])
            elif kind == "CLOSEA":
                pAps.release()
                pA.release()
                mid["paux"] = tc.alloc_tile_pool(name="paux", bufs=2,
                                                 space="PSUM")
            elif kind == "CLOSEVTB":
                pvtb.release()
            elif kind == "OPEND":
                emit_opend()
            elif kind == "LT0":
                emit_lt(0)
            elif kind == "LT1A":
                emit_lt(1, 0)
            elif kind == "LT1":
                emit_lt(1, 1)
            elif kind == "WOQ":
                emit_woq(it[1], it[2])
            elif kind == "EV":
                emit_even(it[1], it[2])
            elif kind == "TAIL":
                emit_tail(it[1], it[2])
            elif kind == "FLUSH":
                if state["pending"] is not None:
                    p = state["pending"]
                    rec = norm1(*p)
                    norm2(*p, rec)
                    state["pending"] = None

        def emit_btask(t, fillers):
            _log(f"B{t}")
            sidx, si, j2 = t
            h, b = sections[sidx]
            if j2 == 0:
                state["po"] = pbo.tile([HD, SQ], dt.float32, name="po",
                                       tag="po")
                state["acc"] = pbe.tile([128, SQ], dt.bfloat16, name="acc",
                                        tag="acc")
            po, acc = state["po"], state["acc"]
            psc = psc_of.pop(t)
            et = emit_exp(t, psc)
            emit_acc(t, et, acc)
            # PE fillers go after scores(t+1) so the scores->exp chain is
            # never delayed; av(t)'s exp wait is covered by the filler
            ni = bidx[t] + 1
            if ni < len(tasks):
                _log(f"S{tasks[ni]}")
                emit_scores(tasks[ni])
            for f in fillers:
                run_filler(f)
            if j2 == 0 and state["pending"] is not None:
                p = state["pending"]
                rec = norm1(*p)
                norm2(*p, rec)
                state["pending"] = None
            emit_av(t, et, po)
            if j2 == (si + 1) * DIAG // 2 - 1:   # last pair of q-block
                state["pending"] = (po, acc, h, b, si)

        def _log(label):
            try:
                EMIT_LOG.append((int(nc._state.next_id()), label))
            except Exception:
                pass

        _mark(nc, "A0")
        fillq = []
        primed = False
        tail_now = False
        for it in items:
            if it[0] == "FLUSH":
                tail_now = True
            if it[0] == "B":
                if not primed:
                    _mark(nc, "B")
                    emit_scores(it[1])
                    primed = True
                emit_btask(it[1], fillq)
                fillq = []
            elif (not primed
                  or it[0] in ("FLUSH", "LT1", "TAIL", "EV")
                  and tail_now
                  or (it[0] == "WOQ" and it[1] >= 2)):
                # head items (before the first B task) and tail items (after
                # the last one) run immediately
                if it[0] == "FLUSH":
                    _mark(nc, "D")
                run_filler(it)
            else:
                fillq.append(it)
        for f in fillq:
            run_filler(f)

        # ---------------- release mid-stream pools (LIFO) ----------------
        for name in ("pdot", "pstg", "pdw", "plt"):
            if name in mid:
                mid[name].release()
        if "paux" in mid:
            mid["paux"].release()
        for pool in (pbo, pbsc_b, pbsc_a, pbn, pbd, pbe, qkvp, pp):
            pool.release()

    nc.compile()
    return nc


def _host_prep(x, cos, sin, wq, wk, wv, wo):
    x = np.asarray(x, dtype=np.float32)
    cos = np.asarray(cos, dtype=np.float32)
    sin = np.asarray(sin, dtype=np.float32)
    wq = np.asarray(wq, dtype=np.float32)
    wk = np.asarray(wk, dtype=np.float32)
    wv = np.asarray(wv, dtype=np.float32)
    wo = np.asarray(wo, dtype=np.float32)

    xT = np.ascontiguousarray(x.reshape(BS, D).T.astype(BF16))         # [D, BS]
    cosT = np.ascontiguousarray(cos[0].T)                              # [HD, S]
    sinT = np.ascontiguousarray(sin[0].T).copy()
    sinT[:64] = -sinT[:64]                      # fold rotate_half sign into sin

    # strict-lower -1e9 triangle: lhsT of the diagonal-block mask matmul
    rr = np.arange(128)
    tri = np.where(rr[:, None] < rr[None, :], -1e9, 0.0)
    tri = np.ascontiguousarray(tri.astype(BF16))

    ident = np.eye(128, dtype=np.float32).astype(BF16)
    onesb = np.ones((128, 1), dtype=np.float32).astype(BF16)
    onesf = np.ones((1, 128), dtype=np.float32).astype(BF16)

    wq_bf = wq.astype(BF16)
    wk_bf = wk.astype(BF16)
    wv_bf = wv.astype(BF16)
    wo_bf = np.ascontiguousarray(wo.astype(BF16))

    cos_bf = cosT.astype(BF16)
    sin_bf = sinT.astype(BF16)
    id_cos = np.ones_like(cos_bf)
    id_sin = np.zeros_like(sin_bf)

    in_maps = []
    for c in range(N_CORES):
        kvh = c // 2
        is_k_core = (c % 2 == 0)
        wkv_full = wk_bf if is_k_core else wv_bf
        in_maps.append({
            "xT": xT,
            "cosT": cos_bf,
            "sinTs": sin_bf,
            "coskv": cos_bf if is_k_core else id_cos,
            "sinkv": sin_bf if is_k_core else id_sin,
            "wq": np.ascontiguousarray(wq_bf[:, c * HPC * HD:(c + 1) * HPC * HD]),
            "wkv": np.ascontiguousarray(wkv_full[:, kvh * HD:(kvh + 1) * HD]),
            "wo": wo_bf,
            "tri": tri,
            "ident": ident,
            "onesb": onesb,
            "onesf": onesf,
        })
    return in_maps


def kernel(x, cos, sin, wq, wk, wv, wo):
    from concourse.bass_utils import run_bass_kernel_spmd

    if "nc" not in _CACHE:
        _CACHE["nc"] = _build()
    nc = _CACHE["nc"]

    in_maps = _host_prep(x, cos, sin, wq, wk, wv, wo)
    res = run_bass_kernel_spmd(nc, in_maps, core_ids=list(range(N_CORES)))
    shards = [np.asarray(res.results[c]["out"], dtype=np.float32)
              for c in range(N_CORES)]
    return np.concatenate(shards, axis=0).reshape(B, S, D)


# revision 8
# speedup vs baseline: 1.0087x; 1.0087x over previous
# GQA causal attention with RoPE on 8 TRN2 NeuronCores (tensor-parallel over
# heads) -- fused-pipeline version.
#
# Reference computation (B=2, S=4096, D=2048, H=16 heads, KVH=4 kv heads,
# HD=128): q/k/v projections -> RoPE on q,k -> causal GQA attention -> o_proj.
#
# Sharding (per hint): core c owns Q heads {2c, 2c+1}; kv head c//2 is split
# across the core pair -- the even core projects K (with RoPE), the odd core
# projects V (same SPMD program: its RoPE tables are cos=1/sin=0), and a
# pairwise AllGather swaps the halves per sequence-half.  Attention context
# is produced transposed [HD, B*S] per head; a per-head AllToAll
# redistributes rows for the o_proj row shard.
#
# This version software-pipelines the WHOLE kernel as one flat emission
# stream so the PE (the global bottleneck at ~560us of matmul work) never
# idles:
#   - batch-0 projections run first (PE-solid), shipping each K/V
#     sequence-half to the pair exchange as soon as it completes;
#   - batch-1 projections are chopped into 8-matmul chunks and interleaved
#     one-per-task into attention section (h0,b0) -- the exp latency of task
#     t is hidden behind the chunk emitted before scores(t+1);
#   - the first four q-block rows of (h1,b0) only need the half-0 K/V
#     exchange, so they run as guests inside A0's back half where the
#     scalar engine is idle -- their exp leaves the ACT-paced sections;
#   - the o_proj even-head pass (which only needs the h0 AllToAll, complete
#     at mid-attention) is interleaved into the ACT-bound sections (h1,b0/b1)
#     and staged to SBUF, so only the odd-head pass remains after the last
#     attention task;
#   - PSUM is juggled via dual-side pool stacks: the projection pools live on
#     the right stack and release mid-stream, after which the score pipeline
#     gets its second buffer.
#
# Causal masking costs no vector work: a -1e9 strict-upper-triangle is added
# to diagonal 128-blocks inside the score-PSUM accumulation group by a
# 128-col matmul, so exp underflows to exact 0 there; fully-masked ranges of
# the dedicated diagonal et tiles are memzero'd once.  The softmax
# denominator accumulates per kv-block on the DVE in bf16 and is collapsed
# AND broadcast in one step by a gpsimd partition_all_reduce (the gpsimd
# engine is otherwise idle), so the normalize path has no PE matmuls at all.
# RoPE stages the projection PSUM through bf16 SBUF (scalar-engine copy plus
# two gpsimd partition-swapped copies for rotate_half), which frees the
# projection PSUM ring quickly and lets every RoPE DVE op run same-base at
# the 2x 16-bit rate.  V is transposed to natural layout on the DMA XBAR.
# h1's context AllToAll is split into even/odd sequence-parity halves so the
# even half is exchanged one q-block before attention ends and the o_proj
# tail starts immediately.  Matmul operands are bf16; PSUM f32; the output
# is staged bf16 (host converts to f32).

import math
import sys

for _p in ("/opt/trn_rl_repo",):
    if _p not in sys.path:
        sys.path.insert(0, _p)

import numpy as np
import ml_dtypes

B = 2
S = 4096
D = 2048
H = 16
KVH = 4
HD = 128
N_CORES = 8
BS = B * S                  # 8192 flattened rows
SHARD = BS // N_CORES       # 1024 output rows per core
HPC = H // N_CORES          # 2 q heads per core
SCALE = 1.0 / math.sqrt(HD)

SQ = 512                    # q-block (matmul free dim)
KV = 128                    # kv-block (psum partition dim)
DCH = D // 128              # 16 contraction chunks for the projections
NB = S // SQ                # 8 q-blocks per batch
NKV_B = S // KV             # 32 kv-blocks per batch
DIAG = SQ // KV             # 4 kv-blocks per q-block on the causal diagonal
S2 = S // 2

EV_N = 16                   # o_proj even-pass tiles prestaged during attention

BF16 = ml_dtypes.bfloat16

_CACHE = {}
PHASE_MARKS = []
EMIT_LOG = []


def _mark(nc, phase):
    try:
        PHASE_MARKS.append((phase, int(nc._state.next_id())))
    except Exception:
        pass


def _build(sim_mode=False):
    import concourse.mybir as mybir
    import concourse.tile as tile
    from concourse import bacc

    dt = mybir.dt
    nc = bacc.Bacc("TRN2", target_bir_lowering=False, debug=False,
                   enable_asserts=True, num_devices=N_CORES)

    # ---- external inputs (per-core shards supplied via in_maps) ----
    xT = nc.dram_tensor("xT", [D, BS], dt.bfloat16, kind="ExternalInput")
    cosT = nc.dram_tensor("cosT", [HD, S], dt.bfloat16, kind="ExternalInput")
    sinTs = nc.dram_tensor("sinTs", [HD, S], dt.bfloat16, kind="ExternalInput")
    wq = nc.dram_tensor("wq", [D, HPC * HD], dt.bfloat16, kind="ExternalInput")
    wkv = nc.dram_tensor("wkv", [D, HD], dt.bfloat16, kind="ExternalInput")
    coskv = nc.dram_tensor("coskv", [HD, S], dt.bfloat16, kind="ExternalInput")
    sinkv = nc.dram_tensor("sinkv", [HD, S], dt.bfloat16, kind="ExternalInput")
    wo = nc.dram_tensor("wo", [D, D], dt.bfloat16, kind="ExternalInput")
    tri = nc.dram_tensor("tri", [128, 128], dt.bfloat16, kind="ExternalInput")
    ident = nc.dram_tensor("ident", [128, 128], dt.bfloat16, kind="ExternalInput")
    onesb = nc.dram_tensor("onesb", [128, 1], dt.bfloat16, kind="ExternalInput")
    onesf = nc.dram_tensor("onesf", [1, 128], dt.bfloat16, kind="ExternalInput")

    out = nc.dram_tensor("out", [SHARD, D], dt.bfloat16, kind="ExternalOutput")

    # ---- internal DRAM for the pairwise k/v exchange (per sequence-half) ----
    ktv_dram = [nc.dram_tensor(f"ktv{b}", [2, HD, S2], dt.bfloat16)
                for b in range(B)]
    kv_pair = [nc.dram_tensor(f"kvp{b}", [2, 2, HD, S2], dt.bfloat16)
               for b in range(B)]

    # ---- internal DRAM for the AllToAll: one buffer for h0, and h1 split
    # into even/odd sequence-parity halves so the even collective fires one
    # q-block before the end of attention ----
    ao_in = [nc.dram_tensor(f"ao_in{h}", [N_CORES, HD, SHARD], dt.bfloat16)
             for h in range(HPC)]
    ao_ex = [nc.dram_tensor(f"ao_ex{h}", [N_CORES, HD, SHARD], dt.bfloat16)
             for h in range(HPC)]
    ao1_in = [nc.dram_tensor(f"ao1_in{p}", [N_CORES, HD, SQ], dt.bfloat16)
              for p in range(2)]
    ao1_ex = [nc.dram_tensor(f"ao1_ex{p}", [N_CORES, HD, SQ], dt.bfloat16)
              for p in range(2)]
    if sim_mode:
        ao_ex = ao_in   # single-core TimelineSim: same DMA pattern
        ao1_ex = ao1_in

    with tile.TileContext(nc) as tc:
        # -------- long-lived pools (left stacks) --------
        pp = tc.alloc_tile_pool(name="persist", bufs=1)
        qkvp = tc.alloc_tile_pool(name="qkv", bufs=2)
        pbe = tc.alloc_tile_pool(name="pbe", bufs=5)
        pbd = tc.alloc_tile_pool(name="pbd", bufs=1)
        pbn = tc.alloc_tile_pool(name="pbn", bufs=4)
        pbsc_a = tc.alloc_tile_pool(name="pbsca", bufs=1, space="PSUM")
        pbsc_b = tc.alloc_tile_pool(name="pbscb", bufs=1, space="PSUM")
        pbo = tc.alloc_tile_pool(name="pbo", bufs=2, space="PSUM")
        # -------- projection-phase pools (right stacks; close mid-stream) ----
        # vtb sits below pA on the right stack: pA releases first (after the
        # last RoPE) so the score pipeline gets its second PSUM buffer while
        # the V transposes are still reading vtb.
        pvtb = tc.alloc_tile_pool(name="pvtb", bufs=1, side="right")
        pA = tc.alloc_tile_pool(name="pA", bufs=1, side="right")
        pAps = tc.alloc_tile_pool(name="pAps", bufs=2, side="right",
                                  space="PSUM")

        tri_sb = pp.tile([128, 128], dt.bfloat16, name="tri_sb")
        id_sb = pp.tile([128, 128], dt.bfloat16, name="id_sb")
        ob_sb = pp.tile([128, 1], dt.bfloat16, name="ob_sb")
        of_sb = pp.tile([1, 128], dt.bfloat16, name="of_sb")

        cos_sb = pA.tile([HD, S], dt.bfloat16, name="cos_sb")
        sin_sb = pA.tile([HD, S], dt.bfloat16, name="sin_sb")
        ckv_sb = pA.tile([HD, S], dt.bfloat16, name="ckv_sb")
        skv_sb = pA.tile([HD, S], dt.bfloat16, name="skv_sb")
        wq_sb = pA.tile([128, DCH, HPC * HD], dt.bfloat16, name="wq_sb")
        wkv_sb = pA.tile([128, DCH, HD], dt.bfloat16, name="wkv_sb")
        vtb = pvtb.tile([HD, S], dt.bfloat16, name="vtb")

        # first contraction chunks ahead of the rest so the opening matmuls
        # aren't stuck behind the full weight DMA
        wqr = wq[:].rearrange("(k p) m -> p k m", p=128)
        wkvr = wkv[:].rearrange("(k p) m -> p k m", p=128)
        nc.sync.dma_start(out=wq_sb[:, 0:2, :], in_=wqr[:, 0:2, :])
        nc.sync.dma_start(out=wkv_sb[:, 0:2, :], in_=wkvr[:, 0:2, :])

        qts, kts, vns = {}, {}, {}
        xs_state = {}
        ktv_tiles = {}
        tpose_state = {}
        stg = {}
        woq_t = {}
        lt_all = {}
        mid = {}          # pools opened mid-stream

        mybir_exp = mybir.ActivationFunctionType.Exp

        def get_qt(b):
            if b not in qts:
                qts[b] = qkvp.tile([HD, HPC, S], dt.bfloat16, name=f"qt{b}",
                                   tag="qt")
            return qts[b]

        def get_kt(b):
            if b not in kts:
                kts[b] = qkvp.tile([HD, S], dt.bfloat16, name=f"kt{b}",
                                   tag="kt")
            return kts[b]

        def get_vn(b):
            if b not in vns:
                vns[b] = qkvp.tile([128, NKV_B, HD], dt.bfloat16,
                                   name=f"vn{b}", tag="vn")
            return vns[b]

        def load_xs(b, sj, split=2):
            t = pA.tile([128, DCH, SQ], dt.bfloat16, name="xs", tag="xs",
                        bufs=2)
            xr = xT[:, b * S + sj * SQ:b * S + sj * SQ + SQ].rearrange(
                "(k p) n -> p k n", p=128)
            step = DCH // split
            for h0 in range(0, DCH, step):
                nc.sync.dma_start(out=t[:, h0:h0 + step, :],
                                  in_=xr[:, h0:h0 + step, :])
            xs_state[(b, sj)] = t
            return t

        # ---------------- A: projection chunks ----------------
        # One pass per projection (q-head0 / q-head1 / k-or-v), 16 matmuls
        # into a single [128,SQ] PSUM tile (ring 2), emitted as two 8-matmul
        # chunks so interleaved B tasks see fine-grained PE filler.
        a_psum = {}

        def emit_A(b, si, kind, half):
            if kind == 0 and half == 0:
                if b == 0 and si == 0:
                    # feed the serial DMA queue in the exact order the first
                    # projection passes consume: (weight chunk pair, xs
                    # eighth) pairs in k order, with the rope-table halves
                    # slotted in where the first RoPE needs them
                    t = pA.tile([128, DCH, SQ], dt.bfloat16, name="xs",
                                tag="xs", bufs=2)
                    xr = xT[:, 0:SQ].rearrange("(k p) n -> p k n", p=128)
                    for k8 in range(8):
                        nc.sync.dma_start(out=t[:, 2 * k8:2 * k8 + 2, :],
                                          in_=xr[:, 2 * k8:2 * k8 + 2, :])
                        if k8 < 7:
                            nc.sync.dma_start(
                                out=wq_sb[:, 2 * k8 + 2:2 * k8 + 4, :],
                                in_=wqr[:, 2 * k8 + 2:2 * k8 + 4, :])
                        if k8 == 6:
                            nc.sync.dma_start(out=cos_sb[:, 0:S // 4],
                                              in_=cosT[:, 0:S // 4])
                            nc.sync.dma_start(out=sin_sb[:, 0:S // 4],
                                              in_=sinTs[:, 0:S // 4])
                    xs_state[(0, 0)] = t
                    nc.sync.dma_start(out=wkv_sb[:, 2:DCH, :],
                                      in_=wkvr[:, 2:DCH, :])
                    nc.sync.dma_start(out=ckv_sb[:, 0:S // 4],
                                      in_=coskv[:, 0:S // 4])
                    nc.sync.dma_start(out=skv_sb[:, 0:S // 4],
                                      in_=sinkv[:, 0:S // 4])
                    nc.sync.dma_start(out=tri_sb[:], in_=tri[:])
                    nc.sync.dma_start(out=id_sb[:], in_=ident[:])
                    nc.sync.dma_start(out=ob_sb[:], in_=onesb[:])
                    nc.sync.dma_start(out=of_sb[:], in_=onesf[:])
                    # warm the exp table set while the scalar engine is idle
                    warm = pA.tile([1, 8], dt.bfloat16, name="warm")
                    nc.scalar.activation(warm[:], tri_sb[0:1, 0:8],
                                         mybir_exp, scale=1.0)
                if (b, si) not in xs_state:
                    load_xs(b, si, split=2)
                if b == 0 and si == 1:
                    nc.sync.dma_start(out=cos_sb[:, S // 4:S2],
                                      in_=cosT[:, S // 4:S2])
                    nc.sync.dma_start(out=sin_sb[:, S // 4:S2],
                                      in_=sinTs[:, S // 4:S2])
                    nc.sync.dma_start(out=ckv_sb[:, S // 4:S2],
                                      in_=coskv[:, S // 4:S2])
                    nc.sync.dma_start(out=skv_sb[:, S // 4:S2],
                                      in_=sinkv[:, S // 4:S2])
                if b == 0 and si == 2:
                    nc.sync.dma_start(out=cos_sb[:, S2:S], in_=cosT[:, S2:S])
                    nc.sync.dma_start(out=sin_sb[:, S2:S], in_=sinTs[:, S2:S])
                    nc.sync.dma_start(out=ckv_sb[:, S2:S],
                                      in_=coskv[:, S2:S])
                    nc.sync.dma_start(out=skv_sb[:, S2:S],
                                      in_=sinkv[:, S2:S])
                # prefetch the next activation block
                nxt = (b, si + 1) if si + 1 < NB else (b + 1, 0)
                if nxt[0] < B and nxt not in xs_state:
                    load_xs(*nxt)
            if kind == 2 and half == 0 and si % (NB // 2) == 0:
                ktv_tiles[(b, si // (NB // 2))] = pA.tile(
                    [HD, S2], dt.bfloat16, name=f"ktv{b}", tag="ktv", bufs=2)
            xs = xs_state[(b, si)]
            if half == 0:
                a_psum[(b, si, kind)] = pAps.tile([128, SQ], dt.float32,
                                                  name="pp", tag="pp")
            pt = a_psum[(b, si, kind)]
            for k in range(half * 8, half * 8 + 8):
                if kind < 2:
                    lhsT = wq_sb[:, k, kind * HD:(kind + 1) * HD]
                else:
                    lhsT = wkv_sb[:, k, :]
                nc.tensor.matmul(pt[:], lhsT=lhsT, rhs=xs[:, k, :],
                                 start=(k == 0), stop=(k == DCH - 1))
            if half == 1:
                ph = a_psum.pop((b, si, kind))
                l0 = si * SQ
                if kind < 2:
                    cs, sn = cos_sb, sin_sb
                    dest = get_qt(b)[:, kind, l0:l0 + SQ]
                else:
                    cs, sn = ckv_sb, skv_sb
                    lh = l0 % S2
                    dest = ktv_tiles[(b, si // (NB // 2))][:, lh:lh + SQ]
                # stage the projection through bf16 SBUF: a scalar-engine
                # copy (straight) plus two gpsimd copies (partition-swapped
                # halves for rotate_half -- the DVE cannot read two SBUF
                # operands at different base partitions).  All RoPE DVE ops
                # then run same-base at the 2x 16-bit rate, and the PSUM slot
                # frees without any DVE work.
                phb = pA.tile([128, SQ], dt.bfloat16, name="phb", tag="phb",
                              bufs=3)
                phs = pA.tile([128, SQ], dt.bfloat16, name="phs", tag="phs",
                              bufs=3)
                nc.scalar.copy(out=phb[:], in_=ph[:])
                nc.gpsimd.tensor_copy(out=phs[0:64, :], in_=phb[64:128, :])
                nc.gpsimd.tensor_copy(out=phs[64:128, :], in_=phb[0:64, :])
                t1 = pA.tile([128, SQ], dt.bfloat16, name="t1", tag="t1",
                             bufs=2)
                t2 = pA.tile([128, SQ], dt.bfloat16, name="t2", tag="t2",
                             bufs=2)
                nc.vector.tensor_mul(out=t1[:], in0=phb[:],
                                     in1=cs[:, l0:l0 + SQ])
                nc.vector.tensor_mul(out=t2[:], in0=phs[:],
                                     in1=sn[:, l0:l0 + SQ])
                nc.vector.tensor_add(out=dest, in0=t1[:], in1=t2[:])

        def emit_ship(b, hx):
            # ship a completed sequence-half of this core's k-or-v and start
            # the pair exchange
            nc.sync.dma_start(out=ktv_dram[b][hx], in_=ktv_tiles[(b, hx)])
            if not sim_mode:
                nc.gpsimd.collective_compute(
                    "AllGather", mybir.AluOpType.bypass,
                    replica_groups=[[2 * g, 2 * g + 1]
                                    for g in range(N_CORES // 2)],
                    ins=[ktv_dram[b][hx]],
                    outs=[kv_pair[b][hx]])
            else:
                nc.sync.dma_start(out=kv_pair[b][hx, 0], in_=ktv_dram[b][hx])
                nc.sync.dma_start(out=kv_pair[b][hx, 1], in_=ktv_dram[b][hx])

        def emit_xrb(b, hx, part=2):
            # pull back this core's K half and/or V half from the exchange
            if part in (0, 2):
                nc.sync.dma_start(out=get_kt(b)[:, hx * S2:(hx + 1) * S2],
                                  in_=kv_pair[b][hx, 0])
            if part in (1, 2):
                nc.sync.dma_start(out=vtb[:, hx * S2:(hx + 1) * S2],
                                  in_=kv_pair[b][hx, 1])

        def emit_T(b, c):
            # transpose 8 kv-blocks of V from [HD, kv] to natural [kv, HD]
            # on the DMA XBAR (no PE/ACT/PSUM involvement)
            vnb = get_vn(b)
            nc.sync.dma_start_transpose(
                vnb[:, c * 8:c * 8 + 8, :],
                vtb[:, c * 8 * 128:(c * 8 + 8) * 128])

        # ---------------- B: attention machinery ----------------
        sections = [(h, b) for h in range(HPC) for b in range(B)]
        tasks = []
        for sidx, (h, b) in enumerate(sections):
            for si in range(NB):
                for j2 in range((si + 1) * DIAG // 2):
                    tasks.append((sidx, si, j2))
        # guest tasks: the first four q-block rows of (h1,b0) only need the
        # half-0 K/V exchange, so they run inside A0's window where the
        # scalar engine is otherwise idle -- their exp leaves the ACT-paced
        # sections entirely
        guest = [t for t in tasks if t[0] == 2 and t[1] < 4]
        tasks = guest + [t for t in tasks if t not in guest]
        bidx = {t: i for i, t in enumerate(tasks)}

        # dedicated diagonal-pair et tiles: fully-masked column ranges zeroed
        # ONCE (exp only writes the live ranges)
        etdAs, etdBs = [], []
        for r in range(2):
            etdA = pbd.tile([128, 2 * SQ], dt.bfloat16, name=f"etdA{r}")
            etdB = pbd.tile([128, 2 * SQ], dt.bfloat16, name=f"etdB{r}")
            nc.vector.memzero(etdA[:, SQ:SQ + KV])
            nc.vector.memzero(etdB[:, 0:2 * KV])
            nc.vector.memzero(etdB[:, SQ:SQ + 3 * KV])
            etdAs.append(etdA)
            etdBs.append(etdB)
        POOLS = [pbsc_a, pbsc_b]
        psc_of = {}
        psc_n = [0]

        def emit_scores(t):
            sidx, si, j2 = t
            h, b = sections[sidx]
            pool = POOLS[psc_n[0] % len(POOLS)]
            psc_n[0] += 1
            psc = pool.tile([128, 2 * SQ], dt.float32, name="psc", tag="psc")
            qt, kt = qts[b], kts[b]
            ndiag = si * DIAG
            for jj in range(2):
                j = j2 * 2 + jj
                dd = j - ndiag
                half = jj * SQ
                nc.tensor.matmul(
                    psc[:, half:half + SQ],
                    lhsT=kt[:, j * KV:(j + 1) * KV],
                    rhs=qt[:, h, si * SQ:(si + 1) * SQ],
                    start=True, stop=(dd < 0))
                if dd >= 0:
                    # strict-upper -1e9 on the diagonal block: exp -> exact 0
                    nc.tensor.matmul(
                        psc[:, half + dd * KV:half + (dd + 1) * KV],
                        lhsT=tri_sb[:], rhs=id_sb[:],
                        start=False, stop=True, skip_group_check=True)
            psc_of[t] = psc

        def emit_exp(t, psc):
            sidx, si, j2 = t
            ndiag2 = si * DIAG // 2
            if j2 < ndiag2:
                et = pbe.tile([128, 2 * SQ], dt.bfloat16, name="et", tag="et")
                nc.scalar.activation(et[:], psc[:], mybir_exp, scale=SCALE)
            elif j2 == ndiag2:          # diagonal pair A (dd=0,1)
                et = etdAs[si % 2]
                nc.scalar.activation(et[:, 0:SQ], psc[:, 0:SQ],
                                     mybir_exp, scale=SCALE)
                nc.scalar.activation(et[:, SQ + KV:2 * SQ],
                                     psc[:, SQ + KV:2 * SQ],
                                     mybir_exp, scale=SCALE)
            else:                       # diagonal pair B (dd=2,3)
                et = etdBs[si % 2]
                nc.scalar.activation(et[:, 2 * KV:SQ], psc[:, 2 * KV:SQ],
                                     mybir_exp, scale=SCALE)
                nc.scalar.activation(et[:, SQ + 3 * KV:2 * SQ],
                                     psc[:, SQ + 3 * KV:2 * SQ],
                                     mybir_exp, scale=SCALE)
            return et

        def emit_acc(t, et, acc):
            _, si, j2 = t
            if j2 == 0:
                nc.vector.tensor_add(out=acc[:], in0=et[:, 0:SQ],
                                     in1=et[:, SQ:2 * SQ])
            else:
                nc.vector.tensor_add(out=acc[:], in0=acc[:], in1=et[:, 0:SQ])
                nc.vector.tensor_add(out=acc[:], in0=acc[:],
                                     in1=et[:, SQ:2 * SQ])

        def emit_av(t, et, po):
            sidx, si, j2 = t
            h, b = sections[sidx]
            vn = vns[b]
            nkv = (si + 1) * DIAG
            for jj in range(2):
                j = j2 * 2 + jj
                nc.tensor.matmul(po[:], lhsT=vn[:, j, :],
                                 rhs=et[:, jj * SQ:(jj + 1) * SQ],
                                 start=(j == 0), stop=(j == nkv - 1))

        # deferred normalize: stage1 (ones-matmul + reciprocal) in the next
        # q-block's first pair window, stage2 (broadcast matmul + PSUM-direct
        # multiply + per-q-block aob ship) in the second
        from concourse import bass_isa

        def norm1(po_, acc_, hh, bb, si):
            # partition-ALL-reduce of the bf16 denominator accumulator on the
            # otherwise-idle gpsimd engine: every partition gets the sum, so
            # no ones-matmul and no broadcast matmul are needed
            dall = pbn.tile([128, SQ], dt.float32, name="dall", tag="dall",
                            bufs=2)
            nc.gpsimd.partition_all_reduce(dall[:], acc_[:], channels=128,
                                           reduce_op=bass_isa.ReduceOp.add)
            rec = pbn.tile([128, SQ], dt.float32, name="rec", tag="rec",
                           bufs=2)
            nc.vector.reciprocal(out=rec[:], in_=dall[:])
            return rec

        def norm2(po_, acc_, hh, bb, si, rec):
            aob = pbn.tile([HD, SQ], dt.bfloat16, name="aob", tag="aob",
                           bufs=3)
            nc.vector.tensor_mul(out=aob[:], in0=po_[:], in1=rec[:])
            g0 = bb * S + si * SQ
            if hh == 0:
                nc.sync.dma_start(
                    out=ao_in[0][g0 // SHARD, :, g0 % SHARD:g0 % SHARD + SQ],
                    in_=aob[:])
                if bb == B - 1 and si == NB - 1 and not sim_mode:
                    nc.gpsimd.collective_compute(
                        "AllToAll", mybir.AluOpType.bypass,
                        replica_groups=[list(range(N_CORES))],
                        ins=[ao_in[0][:]],
                        outs=[ao_ex[0][:]])
            else:
                par = si % 2
                nc.sync.dma_start(
                    out=ao1_in[par][bb * 4 + si // 2, :, :], in_=aob[:])
                if bb == B - 1 and si >= NB - 2 and not sim_mode:
                    nc.gpsimd.collective_compute(
                        "AllToAll", mybir.AluOpType.bypass,
                        replica_groups=[list(range(N_CORES))],
                        ins=[ao1_in[par][:]],
                        outs=[ao1_ex[par][:]])

        # ---------------- D: o_proj machinery ----------------
        def emit_opend():
            mid["plt"] = tc.alloc_tile_pool(name="plt", bufs=1)
            mid["pdw"] = tc.alloc_tile_pool(name="pdw", bufs=2)
            mid["pstg"] = tc.alloc_tile_pool(name="pstg", bufs=EV_N)
            mid["pdot"] = tc.alloc_tile_pool(name="pdot", bufs=4)

        def emit_lt(h, part=2):
            if h == 0:
                lt = mid["plt"].tile([128, SHARD // 128, N_CORES, 128],
                                     dt.bfloat16, name="lt0")
                lt_all[h] = lt
                nc.sync.dma_start(
                    out=lt[:],
                    in_=ao_ex[0][:].rearrange("a p (s n) -> p s a n", n=128))
                return
            # per-si chunks from the parity halves: chunks 0-3 (even parity)
            # are exchanged one q-block before the last ship, so their loads
            # are emitted during the last attention q-block
            if 1 not in lt_all:
                lt_all[1] = mid["plt"].tile(
                    [128, SHARD // 128, N_CORES, 128], dt.bfloat16,
                    name="lt1")
            lt = lt_all[1]
            sls = range(0, 4) if part == 0 else range(4, 8)
            for sl in sls:
                par, c0 = sl // 4, (sl % 4) * 128
                nc.sync.dma_start(
                    out=lt[:, sl],
                    in_=ao1_ex[par][:, :, c0:c0 + 128]
                    .rearrange("a p n -> p a n"))

        def emit_woq(dj, split):
            t = mid["pdw"].tile([128, DCH, SQ], dt.bfloat16, name="woq",
                                tag="woq")
            r = wo[:, dj * SQ:(dj + 1) * SQ].rearrange("(k p) m -> p k m",
                                                       p=128)
            if split:
                # even k chunks first (the even-head pass uses k=2j)
                nc.sync.dma_start(out=t[:, 0:DCH:2, :], in_=r[:, 0:DCH:2, :])
                nc.sync.dma_start(out=t[:, 1:DCH:2, :], in_=r[:, 1:DCH:2, :])
            else:
                nc.sync.dma_start(out=t[:], in_=r)
            woq_t[dj] = t

        def emit_even(dj, sl):
            # even-head half of o_proj tile (dj, sl), staged to SBUF f32 so
            # the PSUM bank recycles; the odd pass adds it back in the tail
            pev = mid["paux"].tile([128, SQ], dt.float32, name="pev",
                                   tag="evn")
            for j in range(N_CORES):
                nc.tensor.matmul(pev[:], lhsT=lt_all[0][:, sl, j, :],
                                 rhs=woq_t[dj][:, 2 * j, :],
                                 start=(j == 0), stop=(j == N_CORES - 1))
            st = mid["pstg"].tile([128, SQ], dt.float32, name="stg",
                                  tag="stg")
            nc.vector.tensor_copy(out=st[:], in_=pev[:])
            stg[(dj, sl)] = st

        tail_n = [0]

        def emit_tail(dj, sl):
            # odd-head pass (+ even remainder) for o_proj tile (dj, sl);
            # po slots alternate between the two PSUM pools (ring 4) so the
            # tail is never blocked behind the final normalize chain
            tail_n[0] += 1
            if tail_n[0] % 2 == 0:
                pod = mid["paux"].tile([HD, SQ], dt.float32, name="po",
                                       tag="evn")
            else:
                pod = pbo.tile([HD, SQ], dt.float32, name="po", tag="po")
            pre = stg.get((dj, sl))
            hps = (1,) if pre is not None else (0, 1)
            for hp in hps:
                for j in range(N_CORES):
                    nc.tensor.matmul(
                        pod[:], lhsT=lt_all[hp][:, sl, j, :],
                        rhs=woq_t[dj][:, 2 * j + hp, :],
                        start=(hp == hps[0] and j == 0),
                        stop=(hp == hps[-1] and j == N_CORES - 1))
            ot = mid["pdot"].tile([128, SQ], dt.bfloat16, name="ot", tag="ot")
            if pre is not None:
                nc.vector.tensor_add(out=ot[:], in0=pod[:], in1=pre[:])
            else:
                nc.scalar.copy(out=ot[:], in_=pod[:])
            nc.sync.dma_start(
                out=out[sl * 128:(sl + 1) * 128, dj * SQ:(dj + 1) * SQ],
                in_=ot[:])

        # ---------------- schedule assembly ----------------
        a_chunks = {b: [(b, si, kind, half)
                        for si in range(NB)
                        for kind in range(3)
                        for half in range(2)]
                    for b in range(B)}

        items = []
        # batch-0 projections, PE-solid, shipping halves as they complete;
        # once the half-0 exchange is back (si4) the V half transposes land
        # and the guest (h1,b0) tasks interleave with the si5-7 chunks
        gi = 0
        for ch in a_chunks[0]:
            b, si, kind, half = ch
            if si == 5 and kind == 0 and half == 0:
                items.append(("T", 0, 0))
                items.append(("T", 0, 1))
            items.append(("A",) + ch)
            if si >= 5 and gi < len(guest):
                items.append(("B", guest[gi]))
                gi += 1
            if kind == 2 and half == 1 and si in (NB // 2 - 1, NB - 1):
                items.append(("SHIP", 0, si // (NB // 2)))
                items.append(("XRB", 0, si // (NB // 2)))
        while gi < len(guest):
            items.append(("B", guest[gi]))
            gi += 1

        # section (0,0) with batch-1 projection chunks interleaved one per
        # task (done by task 48), then T(b1) and the psc second buffer
        sec_tasks = [[t for t in tasks if t[0] == s and (s != 2 or t[1] >= 4)]
                     for s in range(4)]
        a1 = list(a_chunks[1])
        merged = []
        ai = 0
        for ti, t in enumerate(sec_tasks[0]):
            if ai < len(a1):
                merged.append(("A",) + a1[ai])
                b, si, kind, half = a1[ai]
                if kind == 2 and half == 1 and si in (NB // 2 - 1, NB - 1):
                    merged.append(("SHIP", 1, si // (NB // 2)))
                ai += 1
            if ti in (2, 4):
                merged.append(("T", 0, ti // 2 + 1))
            if ti == 27:
                merged.append(("XRB", 1, 0, 0))
            if ti == 29:
                merged.append(("XRB", 1, 0, 1))
            if ti in (32, 34):
                merged.append(("T", 1, (ti - 32) // 2))
            if ti == 48:
                merged.append(("CLOSEA",))
                merged.append(("XRB", 1, 1))
            if ti in (50, 52):
                merged.append(("T", 1, (ti - 50) // 2 + 2))
            if ti == 58:
                merged.append(("CLOSEVTB",))
            merged.append(("B", t))
        items += merged
        # section (0,1): mostly pure attention; prefetch the o_proj weights
        # here where the serial DMA queue is quiet
        for ti, t in enumerate(sec_tasks[1]):
            if ti == 8:
                items.append(("OPEND",))
                items.append(("WOQ", 0, True))
            if ti == 40:
                items.append(("WOQ", 1, False))
            items.append(("B", t))
        # section (1,0): open o_proj pools once h0's AllToAll has fired
        # (inside the norm2 of (0,1)'s last q-block, processed at task 1)
        ev_slots0 = (8, 17, 26, 35, 44)
        done_ev = 0
        for ti, t in enumerate(sec_tasks[2]):
            if ti == 4:
                items.append(("LT0",))
            if ti in ev_slots0 and done_ev < EV_N:
                items.append(("EV", done_ev // 8, done_ev % 8))
                done_ev += 1
            items.append(("B", t))
        # section (1,1): more even-pass tiles in the ACT-bound slack
        ev_slots = (10, 22, 34, 46, 58)
        for ti, t in enumerate(sec_tasks[3]):
            if ti in ev_slots and done_ev < EV_N:
                items.append(("EV", done_ev // 8, done_ev % 8))
                done_ev += 1
            if ti == 66:
                items.append(("LT1A",))
            items.append(("B", t))
        # tail: flush the last norm, land h1 context; the last even-pass
        # tiles (lt0-only) fill the flush->lt1 latency
        items.append(("FLUSH",))
        items.append(("LT1",))
        while done_ev < EV_N:
            items.append(("EV", done_ev // 8, done_ev % 8))
            done_ev += 1
        # even-parity rows first: their h1 exchange fired one q-block early
        for sl in range(4):
            items.append(("TAIL", 0, sl))
        for sl in range(4):
            items.append(("TAIL", 1, sl))
        for sl in range(4, 8):
            items.append(("TAIL", 0, sl))
        items.append(("WOQ", 2, False))
        for sl in range(4, 8):
            items.append(("TAIL", 1, sl))
        items.append(("WOQ", 3, False))
        for sl in range(8):
            items.append(("TAIL", 2, sl))
        for sl in range(8):
            items.append(("TAIL", 3, sl))

        # ---------------- executor ----------------
        state = {"pending": None, "po": None, "acc": None}

        def run_filler(it):
            kind = it[0]
            _log(str(it))
            if kind == "A":
                emit_A(*it[1:])
            elif kind == "SHIP":
                emit_ship(*it[1:])
            elif kind == "XRB":
                emit_xrb(*it[1:])
            elif kind == "T":
                emit_T(*it[1:])
            elif kind == "CLOSEA":
                pAps.release()
                pA.release()
                mid["paux"] = tc.alloc_tile_pool(name="paux", bufs=2,
                                                 space="PSUM")
            elif kind == "CLOSEVTB":
                pvtb.release()
            elif kind == "OPEND":
                emit_opend()
            elif kind == "LT0":
                emit_lt(0)
            elif kind == "LT1A":
                emit_lt(1, 0)
            elif kind == "LT1":
                emit_lt(1, 1)
            elif kind == "WOQ":
                emit_woq(it[1], it[2])
            elif kind == "EV":
                emit_even(it[1], it[2])
            elif kind == "TAIL":
                emit_tail(it[1], it[2])
            elif kind == "FLUSH":
                if state["pending"] is not None:
                    p = state["pending"]
                    rec = norm1(*p)
                    norm2(*p, rec)
                    state["pending"] = None

        def emit_btask(t, fillers):
            _log(f"B{t}")
            sidx, si, j2 = t
            h, b = sections[sidx]
            if j2 == 0:
                state["po"] = pbo.tile([HD, SQ], dt.float32, name="po",
                                       tag="po")
                state["acc"] = pbe.tile([128, SQ], dt.bfloat16, name="acc",
                                        tag="acc")
            po, acc = state["po"], state["acc"]
            psc = psc_of.pop(t)
            et = emit_exp(t, psc)
            emit_acc(t, et, acc)
            # PE fillers go after scores(t+1) so the scores->exp chain is
            # never delayed; av(t)'s exp wait is covered by the filler
            ni = bidx[t] + 1
            if ni < len(tasks):
                _log(f"S{tasks[ni]}")
                emit_scores(tasks[ni])
            for f in fillers:
                run_filler(f)
            if j2 == 0 and state["pending"] is not None:
                p = state["pending"]
                rec = norm1(*p)
                norm2(*p, rec)
                state["pending"] = None
            emit_av(t, et, po)
            if j2 == (si + 1) * DIAG // 2 - 1:   # last pair of q-block
                state["pending"] = (po, acc, h, b, si)

        def _log(label):
            try:
                EMIT_LOG.append((int(nc._state.next_id()), label))
            except Exception:
                pass

        _mark(nc, "A0")
        fillq = []
        primed = False
        tail_now = False
        for it in items:
            if it[0] == "FLUSH":
                tail_now = True
            if it[0] == "B":
                if not primed:
                    _mark(nc, "B")
                    emit_scores(it[1])
                    primed = True
                emit_btask(it[1], fillq)
                fillq = []
            elif (not primed
                  or it[0] in ("FLUSH", "LT1", "TAIL", "EV")
                  and tail_now
                  or (it[0] == "WOQ" and it[1] >= 2)):
                # head items (before the first B task) and tail items (after
                # the last one) run immediately
                if it[0] == "FLUSH":
                    _mark(nc, "D")
                run_filler(it)
            else:
                fillq.append(it)
        for f in fillq:
            run_filler(f)

        # ---------------- release mid-stream pools (LIFO) ----------------
        for name in ("pdot", "pstg", "pdw", "plt"):
            if name in mid:
                mid[name].release()
        if "paux" in mid:
            mid["paux"].release()
        for pool in (pbo, pbsc_b, pbsc_a, pbn, pbd, pbe, qkvp, pp):
            pool.release()

    nc.compile()
    return nc


def _host_prep(x, cos, sin, wq, wk, wv, wo):
    x = np.asarray(x, dtype=np.float32)
    cos = np.asarray(cos, dtype=np.float32)
    sin = np.asarray(sin, dtype=np.float32)
    wq = np.asarray(wq, dtype=np.float32)
    wk = np.asarray(wk, dtype=np.float32)
    wv = np.asarray(wv, dtype=np.float32)
    wo = np.asarray(wo, dtype=np.float32)

    xT = np.ascontiguousarray(x.reshape(BS, D).T.astype(BF16))         # [D, BS]
    cosT = np.ascontiguousarray(cos[0].T)                              # [HD, S]
    sinT = np.ascontiguousarray(sin[0].T).copy()
    sinT[:64] = -sinT[:64]                      # fold rotate_half sign into sin

    # strict-lower -1e9 triangle: lhsT of the diagonal-block mask matmul
    rr = np.arange(128)
    tri = np.where(rr[:, None] < rr[None, :], -1e9, 0.0)
    tri = np.ascontiguousarray(tri.astype(BF16))

    ident = np.eye(128, dtype=np.float32).astype(BF16)
    onesb = np.ones((128, 1), dtype=np.float32).astype(BF16)
    onesf = np.ones((1, 128), dtype=np.float32).astype(BF16)

    wq_bf = wq.astype(BF16)
    wk_bf = wk.astype(BF16)
    wv_bf = wv.astype(BF16)
    wo_bf = np.ascontiguousarray(wo.astype(BF16))

    cos_bf = cosT.astype(BF16)
    sin_bf = sinT.astype(BF16)
    id_cos = np.ones_like(cos_bf)
    id_sin = np.zeros_like(sin_bf)

    in_maps = []
    for c in range(N_CORES):
        kvh = c // 2
        is_k_core = (c % 2 == 0)
        wkv_full = wk_bf if is_k_core else wv_bf
        in_maps.append({
            "xT": xT,
            "cosT": cos_bf,
            "sinTs": sin_bf,
            "coskv": cos_bf if is_k_core else id_cos,
            "sinkv": sin_bf if is_k_core else id_sin,
            "wq": np.ascontiguousarray(wq_bf[:, c * HPC * HD:(c + 1) * HPC * HD]),
            "wkv": np.ascontiguousarray(wkv_full[:, kvh * HD:(kvh + 1) * HD]),
            "wo": wo_bf,
            "tri": tri,
            "ident": ident,
            "onesb": onesb,
            "onesf": onesf,
        })
    return in_maps


def kernel(x, cos, sin, wq, wk, wv, wo):
    from concourse.bass_utils import run_bass_kernel_spmd

    if "nc" not in _CACHE:
        _CACHE["nc"] = _build()
    nc = _CACHE["nc"]

    in_maps = _host_prep(x, cos, sin, wq, wk, wv, wo)
    res = run_bass_kernel_spmd(nc, in_maps, core_ids=list(range(N_CORES)))
    shards = [np.asarray(res.results[c]["out"], dtype=np.float32)
              for c in range(N_CORES)]
    return np.concatenate(shards, axis=0).reshape(B, S, D)


# revision 9
# speedup vs baseline: 1.0122x; 1.0034x over previous
# GQA causal attention with RoPE on 8 TRN2 NeuronCores (tensor-parallel over
# heads) -- fused-pipeline version.
#
# Reference computation (B=2, S=4096, D=2048, H=16 heads, KVH=4 kv heads,
# HD=128): q/k/v projections -> RoPE on q,k -> causal GQA attention -> o_proj.
#
# Sharding (per hint): core c owns Q heads {2c, 2c+1}; kv head c//2 is split
# across the core pair -- the even core projects K (with RoPE), the odd core
# projects V (same SPMD program: its RoPE tables are cos=1/sin=0), and a
# pairwise AllGather swaps the halves per sequence-half.  Attention context
# is produced transposed [HD, B*S] per head; a per-head AllToAll
# redistributes rows for the o_proj row shard.
#
# This version software-pipelines the WHOLE kernel as one flat emission
# stream so the PE (the global bottleneck at ~560us of matmul work) never
# idles:
#   - batch-0 projections run first (PE-solid), shipping each K/V
#     sequence-half to the pair exchange as soon as it completes;
#   - batch-1 projections are chopped into 8-matmul chunks and interleaved
#     one-per-task into attention section (h0,b0) -- the exp latency of task
#     t is hidden behind the chunk emitted before scores(t+1);
#   - the first four q-block rows of (h1,b0) only need the half-0 K/V
#     exchange, so they run as guests inside A0's back half where the
#     scalar engine is idle -- their exp leaves the ACT-paced sections;
#   - the o_proj even-head pass (which only needs the h0 AllToAll, complete
#     at mid-attention) is interleaved into the ACT-bound sections (h1,b0/b1)
#     and staged to SBUF, so only the odd-head pass remains after the last
#     attention task;
#   - PSUM is juggled via dual-side pool stacks: the projection pools live on
#     the right stack and release mid-stream, after which the score pipeline
#     gets its second buffer.
#
# Causal masking costs no vector work: a -1e9 strict-upper-triangle is added
# to diagonal 128-blocks inside the score-PSUM accumulation group by a
# 128-col matmul, so exp underflows to exact 0 there; fully-masked ranges of
# the dedicated diagonal et tiles are memzero'd once.  The softmax
# denominator accumulates per kv-block on the DVE in bf16 and is collapsed
# AND broadcast in one step by a gpsimd partition_all_reduce (the gpsimd
# engine is otherwise idle), so the normalize path has no PE matmuls at all.
# RoPE stages the projection PSUM through bf16 SBUF (scalar-engine copy plus
# two gpsimd partition-swapped copies for rotate_half), which frees the
# projection PSUM ring quickly and lets every RoPE DVE op run same-base at
# the 2x 16-bit rate.  V is transposed to natural layout on the DMA XBAR.
# h1's context AllToAll is split into even/odd sequence-parity halves so the
# even half is exchanged one q-block before attention ends and the o_proj
# tail starts immediately.  Matmul operands are bf16; PSUM f32; the output
# is staged bf16 (host converts to f32).

import math
import sys

for _p in ("/opt/trn_rl_repo",):
    if _p not in sys.path:
        sys.path.insert(0, _p)

import numpy as np
import ml_dtypes

B = 2
S = 4096
D = 2048
H = 16
KVH = 4
HD = 128
N_CORES = 8
BS = B * S                  # 8192 flattened rows
SHARD = BS // N_CORES       # 1024 output rows per core
HPC = H // N_CORES          # 2 q heads per core
SCALE = 1.0 / math.sqrt(HD)

SQ = 512                    # q-block (matmul free dim)
KV = 128                    # kv-block (psum partition dim)
DCH = D // 128              # 16 contraction chunks for the projections
NB = S // SQ                # 8 q-blocks per batch
NKV_B = S // KV             # 32 kv-blocks per batch
DIAG = SQ // KV             # 4 kv-blocks per q-block on the causal diagonal
S2 = S // 2

EV_N = 16                   # o_proj even-pass tiles prestaged during attention

BF16 = ml_dtypes.bfloat16

_CACHE = {}
PHASE_MARKS = []
EMIT_LOG = []


def _mark(nc, phase):
    try:
        PHASE_MARKS.append((phase, int(nc._state.next_id())))
    except Exception:
        pass


def _build(sim_mode=False):
    import concourse.mybir as mybir
    import concourse.tile as tile
    from concourse import bacc

    dt = mybir.dt
    nc = bacc.Bacc("TRN2", target_bir_lowering=False, debug=False,
                   enable_asserts=True, num_devices=N_CORES)

    # ---- external inputs (per-core shards supplied via in_maps) ----
    xT = nc.dram_tensor("xT", [D, BS], dt.bfloat16, kind="ExternalInput")
    cosT = nc.dram_tensor("cosT", [HD, S], dt.bfloat16, kind="ExternalInput")
    sinTs = nc.dram_tensor("sinTs", [HD, S], dt.bfloat16, kind="ExternalInput")
    wq = nc.dram_tensor("wq", [D, HPC * HD], dt.bfloat16, kind="ExternalInput")
    wkv = nc.dram_tensor("wkv", [D, HD], dt.bfloat16, kind="ExternalInput")
    coskv = nc.dram_tensor("coskv", [HD, S], dt.bfloat16, kind="ExternalInput")
    sinkv = nc.dram_tensor("sinkv", [HD, S], dt.bfloat16, kind="ExternalInput")
    wo = nc.dram_tensor("wo", [D, D], dt.bfloat16, kind="ExternalInput")
    tri = nc.dram_tensor("tri", [128, 128], dt.bfloat16, kind="ExternalInput")
    ident = nc.dram_tensor("ident", [128, 128], dt.bfloat16, kind="ExternalInput")
    onesb = nc.dram_tensor("onesb", [128, 1], dt.bfloat16, kind="ExternalInput")
    onesf = nc.dram_tensor("onesf", [1, 128], dt.bfloat16, kind="ExternalInput")

    out = nc.dram_tensor("out", [SHARD, D], dt.bfloat16, kind="ExternalOutput")

    # ---- internal DRAM for the pairwise k/v exchange (per sequence-half) ----
    ktv_dram = [nc.dram_tensor(f"ktv{b}", [2, HD, S2], dt.bfloat16)
                for b in range(B)]
    kv_pair = [nc.dram_tensor(f"kvp{b}", [2, 2, HD, S2], dt.bfloat16)
               for b in range(B)]

    # ---- internal DRAM for the AllToAll: one buffer for h0, and h1 split
    # into even/odd sequence-parity halves so the even collective fires one
    # q-block before the end of attention ----
    ao_in = [nc.dram_tensor(f"ao_in{h}", [N_CORES, HD, SHARD], dt.bfloat16)
             for h in range(HPC)]
    ao_ex = [nc.dram_tensor(f"ao_ex{h}", [N_CORES, HD, SHARD], dt.bfloat16)
             for h in range(HPC)]
    ao1_in = [nc.dram_tensor(f"ao1_in{p}", [N_CORES, HD, SQ], dt.bfloat16)
              for p in range(2)]
    ao1_ex = [nc.dram_tensor(f"ao1_ex{p}", [N_CORES, HD, SQ], dt.bfloat16)
              for p in range(2)]
    if sim_mode:
        ao_ex = ao_in   # single-core TimelineSim: same DMA pattern
        ao1_ex = ao1_in

    with tile.TileContext(nc) as tc:
        # -------- long-lived pools (left stacks) --------
        pp = tc.alloc_tile_pool(name="persist", bufs=1)
        qkvp = tc.alloc_tile_pool(name="qkv", bufs=2)
        pbe = tc.alloc_tile_pool(name="pbe", bufs=5)
        pbd = tc.alloc_tile_pool(name="pbd", bufs=1)
        pbn = tc.alloc_tile_pool(name="pbn", bufs=4)
        pbsc_a = tc.alloc_tile_pool(name="pbsca", bufs=1, space="PSUM")
        pbsc_b = tc.alloc_tile_pool(name="pbscb", bufs=1, space="PSUM")
        pbo = tc.alloc_tile_pool(name="pbo", bufs=2, space="PSUM")
        # -------- projection-phase pools (right stacks; close mid-stream) ----
        # vtb sits below pA on the right stack: pA releases first (after the
        # last RoPE) so the score pipeline gets its second PSUM buffer while
        # the V transposes are still reading vtb.
        pvtb = tc.alloc_tile_pool(name="pvtb", bufs=1, side="right")
        pA = tc.alloc_tile_pool(name="pA", bufs=1, side="right")
        pAps = tc.alloc_tile_pool(name="pAps", bufs=2, side="right",
                                  space="PSUM")

        tri_sb = pp.tile([128, 128], dt.bfloat16, name="tri_sb")
        id_sb = pp.tile([128, 128], dt.bfloat16, name="id_sb")
        ob_sb = pp.tile([128, 1], dt.bfloat16, name="ob_sb")
        of_sb = pp.tile([1, 128], dt.bfloat16, name="of_sb")

        cos_sb = pA.tile([HD, S], dt.bfloat16, name="cos_sb")
        sin_sb = pA.tile([HD, S], dt.bfloat16, name="sin_sb")
        ckv_sb = pA.tile([HD, S], dt.bfloat16, name="ckv_sb")
        skv_sb = pA.tile([HD, S], dt.bfloat16, name="skv_sb")
        wq_sb = pA.tile([128, DCH, HPC * HD], dt.bfloat16, name="wq_sb")
        wkv_sb = pA.tile([128, DCH, HD], dt.bfloat16, name="wkv_sb")
        vtb = pvtb.tile([HD, S], dt.bfloat16, name="vtb")

        # first contraction chunks ahead of the rest so the opening matmuls
        # aren't stuck behind the full weight DMA
        wqr = wq[:].rearrange("(k p) m -> p k m", p=128)
        wkvr = wkv[:].rearrange("(k p) m -> p k m", p=128)
        nc.sync.dma_start(out=wq_sb[:, 0:2, :], in_=wqr[:, 0:2, :])
        nc.sync.dma_start(out=wkv_sb[:, 0:2, :], in_=wkvr[:, 0:2, :])

        qts, kts, vns = {}, {}, {}
        xs_state = {}
        ktv_tiles = {}
        tpose_state = {}
        stg = {}
        woq_t = {}
        lt_all = {}
        mid = {}          # pools opened mid-stream

        mybir_exp = mybir.ActivationFunctionType.Exp

        def get_qt(b):
            if b not in qts:
                qts[b] = qkvp.tile([HD, HPC, S], dt.bfloat16, name=f"qt{b}",
                                   tag="qt")
            return qts[b]

        def get_kt(b):
            if b not in kts:
                kts[b] = qkvp.tile([HD, S], dt.bfloat16, name=f"kt{b}",
                                   tag="kt")
            return kts[b]

        def get_vn(b):
            if b not in vns:
                vns[b] = qkvp.tile([128, NKV_B, HD], dt.bfloat16,
                                   name=f"vn{b}", tag="vn")
            return vns[b]

        def load_xs(b, sj, split=2):
            t = pA.tile([128, DCH, SQ], dt.bfloat16, name="xs", tag="xs",
                        bufs=2)
            xr = xT[:, b * S + sj * SQ:b * S + sj * SQ + SQ].rearrange(
                "(k p) n -> p k n", p=128)
            step = DCH // split
            for h0 in range(0, DCH, step):
                nc.sync.dma_start(out=t[:, h0:h0 + step, :],
                                  in_=xr[:, h0:h0 + step, :])
            xs_state[(b, sj)] = t
            return t

        # ---------------- A: projection chunks ----------------
        # One pass per projection (q-head0 / q-head1 / k-or-v), 16 matmuls
        # into a single [128,SQ] PSUM tile (ring 2), emitted as two 8-matmul
        # chunks so interleaved B tasks see fine-grained PE filler.
        a_psum = {}

        def emit_A(b, si, kind, piece):
            np_ = 2 if b == 0 else 4
            kstep = DCH // np_
            if kind == 0 and piece == 0:
                if b == 0 and si == 0:
                    # feed the serial DMA queue in the exact order the first
                    # projection passes consume: (weight chunk pair, xs
                    # eighth) pairs in k order, with the rope-table halves
                    # slotted in where the first RoPE needs them
                    t = pA.tile([128, DCH, SQ], dt.bfloat16, name="xs",
                                tag="xs", bufs=2)
                    xr = xT[:, 0:SQ].rearrange("(k p) n -> p k n", p=128)
                    for k8 in range(8):
                        nc.sync.dma_start(out=t[:, 2 * k8:2 * k8 + 2, :],
                                          in_=xr[:, 2 * k8:2 * k8 + 2, :])
                        if k8 < 7:
                            nc.sync.dma_start(
                                out=wq_sb[:, 2 * k8 + 2:2 * k8 + 4, :],
                                in_=wqr[:, 2 * k8 + 2:2 * k8 + 4, :])
                        if k8 == 6:
                            nc.sync.dma_start(out=cos_sb[:, 0:S // 4],
                                              in_=cosT[:, 0:S // 4])
                            nc.sync.dma_start(out=sin_sb[:, 0:S // 4],
                                              in_=sinTs[:, 0:S // 4])
                    xs_state[(0, 0)] = t
                    nc.sync.dma_start(out=wkv_sb[:, 2:DCH, :],
                                      in_=wkvr[:, 2:DCH, :])
                    nc.sync.dma_start(out=ckv_sb[:, 0:S // 4],
                                      in_=coskv[:, 0:S // 4])
                    nc.sync.dma_start(out=skv_sb[:, 0:S // 4],
                                      in_=sinkv[:, 0:S // 4])
                    nc.sync.dma_start(out=tri_sb[:], in_=tri[:])
                    nc.sync.dma_start(out=id_sb[:], in_=ident[:])
                    nc.sync.dma_start(out=ob_sb[:], in_=onesb[:])
                    nc.sync.dma_start(out=of_sb[:], in_=onesf[:])
                    # warm the exp table set while the scalar engine is idle
                    warm = pA.tile([1, 8], dt.bfloat16, name="warm")
                    nc.scalar.activation(warm[:], tri_sb[0:1, 0:8],
                                         mybir_exp, scale=1.0)
                if (b, si) not in xs_state:
                    load_xs(b, si, split=2)
                if b == 0 and si == 1:
                    nc.sync.dma_start(out=cos_sb[:, S // 4:S2],
                                      in_=cosT[:, S // 4:S2])
                    nc.sync.dma_start(out=sin_sb[:, S // 4:S2],
                                      in_=sinTs[:, S // 4:S2])
                    nc.sync.dma_start(out=ckv_sb[:, S // 4:S2],
                                      in_=coskv[:, S // 4:S2])
                    nc.sync.dma_start(out=skv_sb[:, S // 4:S2],
                                      in_=sinkv[:, S // 4:S2])
                if b == 0 and si == 2:
                    nc.sync.dma_start(out=cos_sb[:, S2:S], in_=cosT[:, S2:S])
                    nc.sync.dma_start(out=sin_sb[:, S2:S], in_=sinTs[:, S2:S])
                    nc.sync.dma_start(out=ckv_sb[:, S2:S],
                                      in_=coskv[:, S2:S])
                    nc.sync.dma_start(out=skv_sb[:, S2:S],
                                      in_=sinkv[:, S2:S])
                # prefetch the next activation block
                nxt = (b, si + 1) if si + 1 < NB else (b + 1, 0)
                if nxt[0] < B and nxt not in xs_state:
                    load_xs(*nxt)
            if kind == 2 and piece == 0 and si % (NB // 2) == 0:
                ktv_tiles[(b, si // (NB // 2))] = pA.tile(
                    [HD, S2], dt.bfloat16, name=f"ktv{b}", tag="ktv", bufs=2)
            xs = xs_state[(b, si)]
            if piece == 0:
                a_psum[(b, si, kind)] = pAps.tile([128, SQ], dt.float32,
                                                  name="pp", tag="pp")
            pt = a_psum[(b, si, kind)]
            for k in range(piece * kstep, piece * kstep + kstep):
                if kind < 2:
                    lhsT = wq_sb[:, k, kind * HD:(kind + 1) * HD]
                else:
                    lhsT = wkv_sb[:, k, :]
                nc.tensor.matmul(pt[:], lhsT=lhsT, rhs=xs[:, k, :],
                                 start=(k == 0), stop=(k == DCH - 1))
            if piece == np_ - 1:
                ph = a_psum.pop((b, si, kind))
                l0 = si * SQ
                if kind < 2:
                    cs, sn = cos_sb, sin_sb
                    dest = get_qt(b)[:, kind, l0:l0 + SQ]
                else:
                    cs, sn = ckv_sb, skv_sb
                    lh = l0 % S2
                    dest = ktv_tiles[(b, si // (NB // 2))][:, lh:lh + SQ]
                # stage the projection through bf16 SBUF: a scalar-engine
                # copy (straight) plus two gpsimd copies (partition-swapped
                # halves for rotate_half -- the DVE cannot read two SBUF
                # operands at different base partitions).  All RoPE DVE ops
                # then run same-base at the 2x 16-bit rate, and the PSUM slot
                # frees without any DVE work.
                phb = pA.tile([128, SQ], dt.bfloat16, name="phb", tag="phb",
                              bufs=3)
                phs = pA.tile([128, SQ], dt.bfloat16, name="phs", tag="phs",
                              bufs=3)
                nc.scalar.copy(out=phb[:], in_=ph[:])
                nc.gpsimd.tensor_copy(out=phs[0:64, :], in_=phb[64:128, :])
                nc.gpsimd.tensor_copy(out=phs[64:128, :], in_=phb[0:64, :])
                t1 = pA.tile([128, SQ], dt.bfloat16, name="t1", tag="t1",
                             bufs=2)
                t2 = pA.tile([128, SQ], dt.bfloat16, name="t2", tag="t2",
                             bufs=2)
                nc.vector.tensor_mul(out=t1[:], in0=phb[:],
                                     in1=cs[:, l0:l0 + SQ])
                nc.vector.tensor_mul(out=t2[:], in0=phs[:],
                                     in1=sn[:, l0:l0 + SQ])
                nc.vector.tensor_add(out=dest, in0=t1[:], in1=t2[:])

        def emit_ship(b, hx):
            # ship a completed sequence-half of this core's k-or-v and start
            # the pair exchange
            nc.sync.dma_start(out=ktv_dram[b][hx], in_=ktv_tiles[(b, hx)])
            if not sim_mode:
                nc.gpsimd.collective_compute(
                    "AllGather", mybir.AluOpType.bypass,
                    replica_groups=[[2 * g, 2 * g + 1]
                                    for g in range(N_CORES // 2)],
                    ins=[ktv_dram[b][hx]],
                    outs=[kv_pair[b][hx]])
            else:
                nc.sync.dma_start(out=kv_pair[b][hx, 0], in_=ktv_dram[b][hx])
                nc.sync.dma_start(out=kv_pair[b][hx, 1], in_=ktv_dram[b][hx])

        def emit_xrb(b, hx, part=2):
            # pull back this core's K half and/or V half from the exchange
            if part in (0, 2):
                nc.sync.dma_start(out=get_kt(b)[:, hx * S2:(hx + 1) * S2],
                                  in_=kv_pair[b][hx, 0])
            if part in (1, 2):
                nc.sync.dma_start(out=vtb[:, hx * S2:(hx + 1) * S2],
                                  in_=kv_pair[b][hx, 1])

        def emit_T(b, c):
            # transpose 8 kv-blocks of V from [HD, kv] to natural [kv, HD]
            # on the DMA XBAR (no PE/ACT/PSUM involvement)
            vnb = get_vn(b)
            nc.sync.dma_start_transpose(
                vnb[:, c * 8:c * 8 + 8, :],
                vtb[:, c * 8 * 128:(c * 8 + 8) * 128])

        # ---------------- B: attention machinery ----------------
        sections = [(h, b) for h in range(HPC) for b in range(B)]
        tasks = []
        for sidx, (h, b) in enumerate(sections):
            for si in range(NB):
                for j2 in range((si + 1) * DIAG // 2):
                    tasks.append((sidx, si, j2))
        # guest tasks: the first four q-block rows of (h1,b0) only need the
        # half-0 K/V exchange, so they run inside A0's window where the
        # scalar engine is otherwise idle -- their exp leaves the ACT-paced
        # sections entirely
        guest = [t for t in tasks if t[0] == 2 and t[1] < 4]
        tasks = guest + [t for t in tasks if t not in guest]
        bidx = {t: i for i, t in enumerate(tasks)}

        # dedicated diagonal-pair et tiles: fully-masked column ranges zeroed
        # ONCE (exp only writes the live ranges)
        etdAs, etdBs = [], []
        for r in range(2):
            etdA = pbd.tile([128, 2 * SQ], dt.bfloat16, name=f"etdA{r}")
            etdB = pbd.tile([128, 2 * SQ], dt.bfloat16, name=f"etdB{r}")
            nc.vector.memzero(etdA[:, SQ:SQ + KV])
            nc.vector.memzero(etdB[:, 0:2 * KV])
            nc.vector.memzero(etdB[:, SQ:SQ + 3 * KV])
            etdAs.append(etdA)
            etdBs.append(etdB)
        POOLS = [pbsc_a, pbsc_b]
        psc_of = {}
        psc_n = [0]

        def emit_scores(t):
            sidx, si, j2 = t
            h, b = sections[sidx]
            pool = POOLS[psc_n[0] % len(POOLS)]
            psc_n[0] += 1
            psc = pool.tile([128, 2 * SQ], dt.float32, name="psc", tag="psc")
            qt, kt = qts[b], kts[b]
            ndiag = si * DIAG
            for jj in range(2):
                j = j2 * 2 + jj
                dd = j - ndiag
                half = jj * SQ
                nc.tensor.matmul(
                    psc[:, half:half + SQ],
                    lhsT=kt[:, j * KV:(j + 1) * KV],
                    rhs=qt[:, h, si * SQ:(si + 1) * SQ],
                    start=True, stop=(dd < 0))
                if dd >= 0:
                    # strict-upper -1e9 on the diagonal block: exp -> exact 0
                    nc.tensor.matmul(
                        psc[:, half + dd * KV:half + (dd + 1) * KV],
                        lhsT=tri_sb[:], rhs=id_sb[:],
                        start=False, stop=True, skip_group_check=True)
            psc_of[t] = psc

        def emit_exp(t, psc):
            sidx, si, j2 = t
            ndiag2 = si * DIAG // 2
            if j2 < ndiag2:
                et = pbe.tile([128, 2 * SQ], dt.bfloat16, name="et", tag="et")
                nc.scalar.activation(et[:], psc[:], mybir_exp, scale=SCALE)
            elif j2 == ndiag2:          # diagonal pair A (dd=0,1)
                et = etdAs[si % 2]
                nc.scalar.activation(et[:, 0:SQ], psc[:, 0:SQ],
                                     mybir_exp, scale=SCALE)
                nc.scalar.activation(et[:, SQ + KV:2 * SQ],
                                     psc[:, SQ + KV:2 * SQ],
                                     mybir_exp, scale=SCALE)
            else:                       # diagonal pair B (dd=2,3)
                et = etdBs[si % 2]
                nc.scalar.activation(et[:, 2 * KV:SQ], psc[:, 2 * KV:SQ],
                                     mybir_exp, scale=SCALE)
                nc.scalar.activation(et[:, SQ + 3 * KV:2 * SQ],
                                     psc[:, SQ + 3 * KV:2 * SQ],
                                     mybir_exp, scale=SCALE)
            return et

        def emit_acc(t, et, acc):
            _, si, j2 = t
            if j2 == 0:
                nc.vector.tensor_add(out=acc[:], in0=et[:, 0:SQ],
                                     in1=et[:, SQ:2 * SQ])
            else:
                nc.vector.tensor_add(out=acc[:], in0=acc[:], in1=et[:, 0:SQ])
                nc.vector.tensor_add(out=acc[:], in0=acc[:],
                                     in1=et[:, SQ:2 * SQ])

        def emit_av(t, et, po):
            sidx, si, j2 = t
            h, b = sections[sidx]
            vn = vns[b]
            nkv = (si + 1) * DIAG
            for jj in range(2):
                j = j2 * 2 + jj
                nc.tensor.matmul(po[:], lhsT=vn[:, j, :],
                                 rhs=et[:, jj * SQ:(jj + 1) * SQ],
                                 start=(j == 0), stop=(j == nkv - 1))

        # deferred normalize: stage1 (ones-matmul + reciprocal) in the next
        # q-block's first pair window, stage2 (broadcast matmul + PSUM-direct
        # multiply + per-q-block aob ship) in the second
        from concourse import bass_isa

        def norm1(po_, acc_, hh, bb, si):
            # partition-ALL-reduce of the bf16 denominator accumulator on the
            # otherwise-idle gpsimd engine: every partition gets the sum, so
            # no ones-matmul and no broadcast matmul are needed
            dall = pbn.tile([128, SQ], dt.float32, name="dall", tag="dall",
                            bufs=2)
            nc.gpsimd.partition_all_reduce(dall[:], acc_[:], channels=128,
                                           reduce_op=bass_isa.ReduceOp.add)
            rec = pbn.tile([128, SQ], dt.float32, name="rec", tag="rec",
                           bufs=2)
            nc.vector.reciprocal(out=rec[:], in_=dall[:])
            return rec

        def norm2(po_, acc_, hh, bb, si, rec):
            aob = pbn.tile([HD, SQ], dt.bfloat16, name="aob", tag="aob",
                           bufs=3)
            nc.vector.tensor_mul(out=aob[:], in0=po_[:], in1=rec[:])
            g0 = bb * S + si * SQ
            if hh == 0:
                nc.sync.dma_start(
                    out=ao_in[0][g0 // SHARD, :, g0 % SHARD:g0 % SHARD + SQ],
                    in_=aob[:])
                if bb == B - 1 and si == NB - 1 and not sim_mode:
                    nc.gpsimd.collective_compute(
                        "AllToAll", mybir.AluOpType.bypass,
                        replica_groups=[list(range(N_CORES))],
                        ins=[ao_in[0][:]],
                        outs=[ao_ex[0][:]])
            else:
                par = si % 2
                nc.sync.dma_start(
                    out=ao1_in[par][bb * 4 + si // 2, :, :], in_=aob[:])
                if bb == B - 1 and si >= NB - 2 and not sim_mode:
                    nc.gpsimd.collective_compute(
                        "AllToAll", mybir.AluOpType.bypass,
                        replica_groups=[list(range(N_CORES))],
                        ins=[ao1_in[par][:]],
                        outs=[ao1_ex[par][:]])

        # ---------------- D: o_proj machinery ----------------
        def emit_opend():
            mid["plt"] = tc.alloc_tile_pool(name="plt", bufs=1)
            mid["pdw"] = tc.alloc_tile_pool(name="pdw", bufs=2)
            mid["pstg"] = tc.alloc_tile_pool(name="pstg", bufs=EV_N)
            mid["pdot"] = tc.alloc_tile_pool(name="pdot", bufs=4)

        def emit_lt(h, part=2):
            if h == 0:
                lt = mid["plt"].tile([128, SHARD // 128, N_CORES, 128],
                                     dt.bfloat16, name="lt0")
                lt_all[h] = lt
                nc.sync.dma_start(
                    out=lt[:],
                    in_=ao_ex[0][:].rearrange("a p (s n) -> p s a n", n=128))
                return
            # per-si chunks from the parity halves: chunks 0-3 (even parity)
            # are exchanged one q-block before the last ship, so their loads
            # are emitted during the last attention q-block
            if 1 not in lt_all:
                lt_all[1] = mid["plt"].tile(
                    [128, SHARD // 128, N_CORES, 128], dt.bfloat16,
                    name="lt1")
            lt = lt_all[1]
            sls = range(0, 4) if part == 0 else range(4, 8)
            for sl in sls:
                par, c0 = sl // 4, (sl % 4) * 128
                nc.sync.dma_start(
                    out=lt[:, sl],
                    in_=ao1_ex[par][:, :, c0:c0 + 128]
                    .rearrange("a p n -> p a n"))

        def emit_woq(dj, split):
            t = mid["pdw"].tile([128, DCH, SQ], dt.bfloat16, name="woq",
                                tag="woq")
            r = wo[:, dj * SQ:(dj + 1) * SQ].rearrange("(k p) m -> p k m",
                                                       p=128)
            if split:
                # even k chunks first (the even-head pass uses k=2j)
                nc.sync.dma_start(out=t[:, 0:DCH:2, :], in_=r[:, 0:DCH:2, :])
                nc.sync.dma_start(out=t[:, 1:DCH:2, :], in_=r[:, 1:DCH:2, :])
            else:
                nc.sync.dma_start(out=t[:], in_=r)
            woq_t[dj] = t

        def emit_even(dj, sl):
            # even-head half of o_proj tile (dj, sl), staged to SBUF f32 so
            # the PSUM bank recycles; the odd pass adds it back in the tail
            pev = mid["paux"].tile([128, SQ], dt.float32, name="pev",
                                   tag="evn")
            for j in range(N_CORES):
                nc.tensor.matmul(pev[:], lhsT=lt_all[0][:, sl, j, :],
                                 rhs=woq_t[dj][:, 2 * j, :],
                                 start=(j == 0), stop=(j == N_CORES - 1))
            st = mid["pstg"].tile([128, SQ], dt.float32, name="stg",
                                  tag="stg")
            nc.vector.tensor_copy(out=st[:], in_=pev[:])
            stg[(dj, sl)] = st

        tail_n = [0]

        def emit_tail(dj, sl):
            # odd-head pass (+ even remainder) for o_proj tile (dj, sl);
            # po slots alternate between the two PSUM pools (ring 4) so the
            # tail is never blocked behind the final normalize chain
            tail_n[0] += 1
            if tail_n[0] % 2 == 0:
                pod = mid["paux"].tile([HD, SQ], dt.float32, name="po",
                                       tag="evn")
            else:
                pod = pbo.tile([HD, SQ], dt.float32, name="po", tag="po")
            pre = stg.get((dj, sl))
            hps = (1,) if pre is not None else (0, 1)
            for hp in hps:
                for j in range(N_CORES):
                    nc.tensor.matmul(
                        pod[:], lhsT=lt_all[hp][:, sl, j, :],
                        rhs=woq_t[dj][:, 2 * j + hp, :],
                        start=(hp == hps[0] and j == 0),
                        stop=(hp == hps[-1] and j == N_CORES - 1))
            ot = mid["pdot"].tile([128, SQ], dt.bfloat16, name="ot", tag="ot")
            if pre is not None:
                nc.vector.tensor_add(out=ot[:], in0=pod[:], in1=pre[:])
            else:
                nc.scalar.copy(out=ot[:], in_=pod[:])
            nc.sync.dma_start(
                out=out[sl * 128:(sl + 1) * 128, dj * SQ:(dj + 1) * SQ],
                in_=ot[:])

        # ---------------- schedule assembly ----------------
        a_chunks = {b: [(b, si, kind, piece)
                        for si in range(NB)
                        for kind in range(3)
                        for piece in range(2 if b == 0 else 4)]
                    for b in range(B)}

        items = []
        # batch-0 projections, PE-solid, shipping halves as they complete;
        # once the half-0 exchange is back (si4) the V half transposes land
        # and the guest (h1,b0) tasks interleave with the si5-7 chunks
        gi = 0
        for ch in a_chunks[0]:
            b, si, kind, half = ch
            if si == 5 and kind == 0 and half == 0:
                items.append(("T", 0, 0))
                items.append(("T", 0, 1))
            items.append(("A",) + ch)
            if si >= 5 and gi < len(guest):
                items.append(("B", guest[gi]))
                gi += 1
            if kind == 2 and half == 1 and si in (NB // 2 - 1, NB - 1):
                items.append(("SHIP", 0, si // (NB // 2)))
                items.append(("XRB", 0, si // (NB // 2)))
        while gi < len(guest):
            items.append(("B", guest[gi]))
            gi += 1

        # section (0,0) with batch-1 projection chunks interleaved one per
        # task (done by task 48), then T(b1) and the psc second buffer
        sec_tasks = [[t for t in tasks if t[0] == s and (s != 2 or t[1] >= 4)]
                     for s in range(4)]
        a1 = list(a_chunks[1])
        merged = []
        ai = 0
        post_ship = {}
        for ti, t in enumerate(sec_tasks[0]):
            nq = 2 if ti % 3 == 0 else 1
            for _ in range(nq):
                if ai >= len(a1):
                    break
                merged.append(("A",) + a1[ai])
                b, si, kind, piece = a1[ai]
                if kind == 2 and piece == 3 and si in (NB // 2 - 1, NB - 1):
                    merged.append(("SHIP", 1, si // (NB // 2)))
                    post_ship[si // (NB // 2)] = ti
                ai += 1
            if ti in (2, 4):
                merged.append(("T", 0, ti // 2 + 1))
            h0t = post_ship.get(0)
            if h0t is not None:
                if ti == h0t + 2:
                    merged.append(("XRB", 1, 0, 0))
                if ti == h0t + 4:
                    merged.append(("XRB", 1, 0, 1))
                if ti in (h0t + 7, h0t + 9):
                    merged.append(("T", 1, (ti - h0t - 7) // 2))
            merged.append(("B", t))
        items += merged
        # section (0,1): hosts the post-A1 bookkeeping (A1's last quarters
        # now run to the end of the previous section) plus the o_proj weight
        # prefetch where the serial DMA queue is quiet
        for ti, t in enumerate(sec_tasks[1]):
            if ti == 0:
                items.append(("CLOSEA",))
                items.append(("XRB", 1, 1))
            if ti in (2, 4):
                items.append(("T", 1, (ti - 2) // 2 + 2))
            if ti == 6:
                items.append(("CLOSEVTB",))
            if ti == 8:
                items.append(("OPEND",))
                items.append(("WOQ", 0, True))
            if ti == 40:
                items.append(("WOQ", 1, False))
            items.append(("B", t))
        # section (1,0): open o_proj pools once h0's AllToAll has fired
        # (inside the norm2 of (0,1)'s last q-block, processed at task 1)
        ev_slots0 = (8, 17, 26, 35, 44)
        done_ev = 0
        for ti, t in enumerate(sec_tasks[2]):
            if ti == 4:
                items.append(("LT0",))
            if ti in ev_slots0 and done_ev < EV_N:
                items.append(("EV", done_ev // 8, done_ev % 8))
                done_ev += 1
            items.append(("B", t))
        # section (1,1): more even-pass tiles in the ACT-bound slack
        ev_slots = (10, 22, 34, 46, 58)
        for ti, t in enumerate(sec_tasks[3]):
            if ti in ev_slots and done_ev < EV_N:
                items.append(("EV", done_ev // 8, done_ev % 8))
                done_ev += 1
            if ti == 66:
                items.append(("LT1A",))
            items.append(("B", t))
        # tail: flush the last norm, land h1 context; the last even-pass
        # tiles (lt0-only) fill the flush->lt1 latency
        items.append(("FLUSH",))
        items.append(("LT1",))
        while done_ev < EV_N:
            items.append(("EV", done_ev // 8, done_ev % 8))
            done_ev += 1
        # even-parity rows first: their h1 exchange fired one q-block early
        for sl in range(4):
            items.append(("TAIL", 0, sl))
        for sl in range(4):
            items.append(("TAIL", 1, sl))
        for sl in range(4, 8):
            items.append(("TAIL", 0, sl))
        items.append(("WOQ", 2, False))
        for sl in range(4, 8):
            items.append(("TAIL", 1, sl))
        items.append(("WOQ", 3, False))
        for sl in range(8):
            items.append(("TAIL", 2, sl))
        for sl in range(8):
            items.append(("TAIL", 3, sl))

        # ---------------- executor ----------------
        state = {"pending": None, "po": None, "acc": None}

        def run_filler(it):
            kind = it[0]
            _log(str(it))
            if kind == "A":
                emit_A(*it[1:])
            elif kind == "SHIP":
                emit_ship(*it[1:])
            elif kind == "XRB":
                emit_xrb(*it[1:])
            elif kind == "T":
                emit_T(*it[1:])
            elif kind == "CLOSEA":
                pAps.release()
                pA.release()
                mid["paux"] = tc.alloc_tile_pool(name="paux", bufs=2,
                                                 space="PSUM")
            elif kind == "CLOSEVTB":
                pvtb.release()
            elif kind == "OPEND":
                emit_opend()
            elif kind == "LT0":
                emit_lt(0)
            elif kind == "LT1A":
                emit_lt(1, 0)
            elif kind == "LT1":
                emit_lt(1, 1)
            elif kind == "WOQ":
                emit_woq(it[1], it[2])
            elif kind == "EV":
                emit_even(it[1], it[2])
            elif kind == "TAIL":
                emit_tail(it[1], it[2])
            elif kind == "FLUSH":
                if state["pending"] is not None:
                    p = state["pending"]
                    rec = norm1(*p)
                    norm2(*p, rec)
                    state["pending"] = None

        def emit_btask(t, fillers):
            _log(f"B{t}")
            sidx, si, j2 = t
            h, b = sections[sidx]
            if j2 == 0:
                state["po"] = pbo.tile([HD, SQ], dt.float32, name="po",
                                       tag="po")
                state["acc"] = pbe.tile([128, SQ], dt.bfloat16, name="acc",
                                        tag="acc")
            po, acc = state["po"], state["acc"]
            psc = psc_of.pop(t)
            et = emit_exp(t, psc)
            emit_acc(t, et, acc)
            # PE fillers go after scores(t+1) so the scores->exp chain is
            # never delayed; av(t)'s exp wait is covered by the filler
            ni = bidx[t] + 1
            if ni < len(tasks):
                _log(f"S{tasks[ni]}")
                emit_scores(tasks[ni])
            for f in fillers:
                run_filler(f)
            if j2 == 0 and state["pending"] is not None:
                p = state["pending"]
                rec = norm1(*p)
                norm2(*p, rec)
                state["pending"] = None
            emit_av(t, et, po)
            if j2 == (si + 1) * DIAG // 2 - 1:   # last pair of q-block
                state["pending"] = (po, acc, h, b, si)

        def _log(label):
            try:
                EMIT_LOG.append((int(nc._state.next_id()), label))
            except Exception:
                pass

        _mark(nc, "A0")
        fillq = []
        primed = False
        tail_now = False
        for it in items:
            if it[0] == "FLUSH":
                tail_now = True
            if it[0] == "B":
                if not primed:
                    _mark(nc, "B")
                    emit_scores(it[1])
                    primed = True
                emit_btask(it[1], fillq)
                fillq = []
            elif (not primed
                  or it[0] in ("FLUSH", "LT1", "TAIL", "EV")
                  and tail_now
                  or (it[0] == "WOQ" and it[1] >= 2)):
                # head items (before the first B task) and tail items (after
                # the last one) run immediately
                if it[0] == "FLUSH":
                    _mark(nc, "D")
                run_filler(it)
            else:
                fillq.append(it)
        for f in fillq:
            run_filler(f)

        # ---------------- release mid-stream pools (LIFO) ----------------
        for name in ("pdot", "pstg", "pdw", "plt"):
            if name in mid:
                mid[name].release()
        if "paux" in mid:
            mid["paux"].release()
        for pool in (pbo, pbsc_b, pbsc_a, pbn, pbd, pbe, qkvp, pp):
            pool.release()

    nc.compile()
    return nc


def _host_prep(x, cos, sin, wq, wk, wv, wo):
    x = np.asarray(x, dtype=np.float32)
    cos = np.asarray(cos, dtype=np.float32)
    sin = np.asarray(sin, dtype=np.float32)
    wq = np.asarray(wq, dtype=np.float32)
    wk = np.asarray(wk, dtype=np.float32)
    wv = np.asarray(wv, dtype=np.float32)
    wo = np.asarray(wo, dtype=np.float32)

    xT = np.ascontiguousarray(x.reshape(BS, D).T.astype(BF16))         # [D, BS]
    cosT = np.ascontiguousarray(cos[0].T)                              # [HD, S]
    sinT = np.ascontiguousarray(sin[0].T).copy()
    sinT[:64] = -sinT[:64]                      # fold rotate_half sign into sin

    # strict-lower -1e9 triangle: lhsT of the diagonal-block mask matmul
    rr = np.arange(128)
    tri = np.where(rr[:, None] < rr[None, :], -1e9, 0.0)
    tri = np.ascontiguousarray(tri.astype(BF16))

    ident = np.eye(128, dtype=np.float32).astype(BF16)
    onesb = np.ones((128, 1), dtype=np.float32).astype(BF16)
    onesf = np.ones((1, 128), dtype=np.float32).astype(BF16)

    wq_bf = wq.astype(BF16)
    wk_bf = wk.astype(BF16)
    wv_bf = wv.astype(BF16)
    wo_bf = np.ascontiguousarray(wo.astype(BF16))

    cos_bf = cosT.astype(BF16)
    sin_bf = sinT.astype(BF16)
    id_cos = np.ones_like(cos_bf)
    id_sin = np.zeros_like(sin_bf)

    in_maps = []
    for c in range(N_CORES):
        kvh = c // 2
        is_k_core = (c % 2 == 0)
        wkv_full = wk_bf if is_k_core else wv_bf
        in_maps.append({
            "xT": xT,
            "cosT": cos_bf,
            "sinTs": sin_bf,
            "coskv": cos_bf if is_k_core else id_cos,
            "sinkv": sin_bf if is_k_core else id_sin,
            "wq": np.ascontiguousarray(wq_bf[:, c * HPC * HD:(c + 1) * HPC * HD]),
            "wkv": np.ascontiguousarray(wkv_full[:, kvh * HD:(kvh + 1) * HD]),
            "wo": wo_bf,
            "tri": tri,
            "ident": ident,
            "onesb": onesb,
            "onesf": onesf,
        })
    return in_maps


def kernel(x, cos, sin, wq, wk, wv, wo):
    from concourse.bass_utils import run_bass_kernel_spmd

    if "nc" not in _CACHE:
        _CACHE["nc"] = _build()
    nc = _CACHE["nc"]

    in_maps = _host_prep(x, cos, sin, wq, wk, wv, wo)
    res = run_bass_kernel_spmd(nc, in_maps, core_ids=list(range(N_CORES)))
    shards = [np.asarray(res.results[c]["out"], dtype=np.float32)
              for c in range(N_CORES)]
    return np.concatenate(shards, axis=0).reshape(B, S, D)
